# revision 59
# baseline (speedup 1.0000x reference)
"""GCN (3x GCNConv+BN+ReLU, linear head) on 8 TRN2 NeuronCores.

Strategy (graph/data parallel, dst-partitioned, bf16 data path):
  - Nodes row-sharded 8 ways; edges partitioned by dst core, grouped by
    dst block (128 nodes), per-block counts aligned across cores (~3% pad)
    so the SPMD program structure is identical on every core.
  - L1: NO device gathers. Host pre-builds an edge-major bf16 feature
    image (already in SBUF layout) streamed sequentially at full DMA
    bandwidth.
  - L2/L3: SWDGE dma_gather (4 queues, bf16 elems) of halo rows from the
    all-gathered activations; self-loops excluded from the gather stream
    and applied per block via a diagonal one-hot matmul reading the
    core-local rows with a sequential DMA.
  - Per 128-edge chunk: one-hot S (iota-compare, DVE, bf16) and a
    segment-sum matmul accumulating in PSUM per 128-node block. Chunks
    are densely packed; a chunk straddling two blocks is applied twice
    with disjoint norm masks.
  - BatchNorm: per-feature sums via ones-matmul + tiny AllReduce; conv
    outputs stay resident in SBUF; affine+ReLU applied in transposed
    layout as one ACT op per tile. Conv bias cancels in train-mode BN.
  - Activations all-gathered between layers in bf16.
"""
import os

import numpy as np

P = 128
NCORES = 8
BANKS = 4
P_CH = 8          # chunks per dma_gather piece
P_CH1 = 32        # chunks per L1 stream-load piece
SE_CH = 64        # S-tile cols per stream-load piece (L2/L3)
EPS = 1e-5


def _pack_idx(idx_flat):
    # [n] int16 -> [128, n//16] wrapped in 16 partitions, replicated x8
    return np.tile(idx_flat.reshape(-1, 16).T, (8, 1)).copy()


# ----------------------------------------------------------------- host prep
def _host_prep(N, src, dst, x32, bf):
    """Build per-core streams/tables + shared static schedule."""
    NPC = N // NCORES
    NBLK = (NPC + P - 1) // P
    BR = -(-N // BANKS)
    D_IN = x32.shape[1]

    loops = np.arange(N, dtype=np.int64)
    src_f = np.concatenate([src, loops])
    dst_f = np.concatenate([dst, loops])
    deg = np.bincount(dst_f, minlength=N).astype(np.float32)
    dinv = np.where(deg > 0, 1.0 / np.sqrt(deg), 0.0).astype(np.float32)
    w_f = (dinv[src_f] * dinv[dst_f]).astype(np.float32)

    # ---------- L1 stream: all edges incl self-loops, grouped by dst block
    core1 = dst_f // NPC
    per1 = []
    for c in range(NCORES):
        sel = np.flatnonzero(core1 == c)
        s, d, ww = src_f[sel], dst_f[sel] - c * NPC, w_f[sel]
        blk = d // P
        o = np.argsort(blk, kind="stable")
        per1.append((s[o], d[o], ww[o], blk[o]))
    cnt1 = np.zeros((NCORES, NBLK), np.int64)
    for c in range(NCORES):
        np.add.at(cnt1[c], per1[c][3], 1)
    m1 = cnt1.max(axis=0)                      # aligned per-block counts
    ofs1 = np.cumsum(m1) - m1
    L1 = int(m1.sum())
    nch1 = -(-L1 // P)
    L1p = nch1 * P

    cols1 = []                                  # (block, chunk)
    parts1 = [[] for _ in range(NBLK)]          # block -> [(chunk, col)]
    for i in range(NBLK):
        if m1[i] == 0:
            continue
        c0 = int(ofs1[i]) // P
        c1 = int(ofs1[i] + m1[i] - 1) // P
        for ch in range(c0, c1 + 1):
            parts1[i].append((ch, len(cols1)))
            cols1.append((i, ch))
    ncol1 = len(cols1)

    # ---------- L2/L3 streams: real edges only, by (dst block, src bank)
    core2 = dst // NPC
    w_r = w_f[: len(src)]
    per2 = []
    for c in range(NCORES):
        sel = np.flatnonzero(core2 == c)
        s, d, ww = src[sel], dst[sel] - c * NPC, w_r[sel]
        blk = d // P
        bank = s // BR
        o = np.argsort(blk * BANKS + bank, kind="stable")
        per2.append((s[o], d[o], ww[o], blk[o], bank[o]))
    cnt2 = np.zeros((NCORES, NBLK, BANKS), np.int64)
    for c in range(NCORES):
        np.add.at(cnt2[c], (per2[c][3], per2[c][4]), 1)
    m2 = cnt2.max(axis=0)                       # [NBLK, BANKS]
    ofs2 = np.cumsum(m2, axis=0) - m2           # stream offset per (blk,bank)
    L2 = m2.sum(axis=0)                         # [BANKS] stream lengths
    nch2 = [int(-(-max(int(L2[b]), 1) // P)) for b in range(BANKS)]

    cols2 = []                                  # (block, bank, chunk) | diag
    parts2 = [[] for _ in range(NBLK)]          # block -> [(bank, chunk, col)]
    diagcol = [0] * NBLK                        # block -> col of its diag tile
    for i in range(NBLK):
        diagcol[i] = len(cols2)
        cols2.append(("diag", i, -1))
        for b in range(BANKS):
            if m2[i, b] == 0:
                continue
            c0 = int(ofs2[i, b]) // P
            c1 = int(ofs2[i, b] + m2[i, b] - 1) // P
            for ch in range(c0, c1 + 1):
                parts2[i].append((b, ch, len(cols2)))
                cols2.append((i, b, ch))
    ncol2 = len(cols2)

    # piece emission schedule (stream pieces to load before block i)
    emit1 = [[] for _ in range(NBLK)]
    seen = set()
    for i in range(NBLK):
        for (ch, _) in parts1[i]:
            pi = ch // P_CH1
            if pi not in seen:
                seen.add(pi)
                emit1[i].append(pi)
    emitQ1 = [[] for _ in range(NBLK)]          # quad one-hot build schedule
    seen = set()
    for i in range(NBLK):
        for (_, col) in parts1[i]:
            qi = col // 4
            if qi not in seen:
                seen.add(qi)
                emitQ1[i].append(qi)
    emit2 = [[] for _ in range(NBLK)]
    seen = set()
    for i in range(NBLK):
        for (b, ch, _) in parts2[i]:
            pi = ch // P_CH
            if (b, pi) not in seen:
                seen.add((b, pi))
                emit2[i].append((b, pi))
    emitS2 = [[] for _ in range(NBLK)]
    seen = set()
    for i in range(NBLK):
        for col in [diagcol[i]] + [c for (_, _, c) in parts2[i]]:
            pi = col // SE_CH
            if pi not in seen:
                seen.add(pi)
                emitS2[i].append(pi)

    # ---------- per-core device arrays
    dev = []
    for c in range(NCORES):
        m = {}
        # L1: edge-major bf16 image [128, nch1*D_IN] + slot/norm cols
        s, d, ww, blk = per1[c]
        pos_in = np.zeros(L1p, np.int64)        # stream pos -> edge idx+1
        cur = 0
        for i in range(NBLK):
            n = int(cnt1[c][i])
            pos_in[int(ofs1[i]):int(ofs1[i]) + n] = np.arange(cur, cur + n) + 1
            cur += n
        srcs = np.zeros(L1p, np.int64)
        valid = pos_in > 0
        srcs[valid] = s[pos_in[valid] - 1]
        dloc = np.zeros(L1p, np.float32)
        wloc = np.zeros(L1p, np.float32)
        dloc[valid] = (d[pos_in[valid] - 1] % P).astype(np.float32)
        wloc[valid] = ww[pos_in[valid] - 1]
        # fold the edge norm into the streamed rows; S becomes a pure one-hot
        xe = (x32[srcs] * wloc[:, None]).astype(bf)   # [L1p, D_IN]
        m["xe"] = np.ascontiguousarray(
            xe.reshape(nch1, P, D_IN).transpose(1, 0, 2).reshape(P, nch1 * D_IN))
        blk_of = np.full(L1p, -1, np.int64)
        for i in range(NBLK):
            blk_of[int(ofs1[i]):int(ofs1[i] + m1[i])] = i
        # masked/pad entries get slot 255: matches no iota value
        slot1 = np.full((P, ncol1), 255.0, np.float32)
        for j, (i, ch) in enumerate(cols1):
            sl = slice(ch * P, (ch + 1) * P)
            mask = (blk_of[sl] == i) & valid[sl]
            slot1[:, j] = np.where(mask, dloc[sl], 255.0)
        m["slot1"] = slot1.astype(bf)

        # L2/L3: idx banks + slot/norm cols
        s, d, ww, blk, bank = per2[c]
        dl_all, w_all, blk_all = [], [], []
        for b in range(BANKS):
            Lb = nch2[b] * P
            idx = np.zeros(Lb, np.int16)
            dl = np.zeros(Lb, np.float32)
            wl = np.zeros(Lb, np.float32)
            bo = np.full(Lb, -1, np.int64)
            vv = np.zeros(Lb, bool)
            selb = bank == b
            sb, db, wb = s[selb], d[selb], ww[selb]
            cpos = 0
            for i in range(NBLK):
                n = int(cnt2[c][i, b])
                o0 = int(ofs2[i, b])
                idx[o0:o0 + n] = (sb[cpos:cpos + n] - b * BR).astype(np.int16)
                dl[o0:o0 + n] = (db[cpos:cpos + n] % P).astype(np.float32)
                wl[o0:o0 + n] = wb[cpos:cpos + n]
                vv[o0:o0 + n] = True
                bo[o0:o0 + int(m2[i, b])] = i
                cpos += n
            dl_all.append(dl)
            w_all.append(wl)
            blk_all.append((bo, vv))
            m[f"idx{b}"] = _pack_idx(idx)
        # per-node dinv columns [128, NBLK]: dinv[src] is folded into the
        # apply-phase row writes, dinv[dst] into the aggregate-copy scale,
        # so the S tiles are PURE one-hots (exactly representable in fp8).
        nod = c * NPC + np.arange(NBLK * P)
        dvl = np.zeros(NBLK * P, np.float32)
        ok = nod < (c + 1) * NPC
        dvl[ok] = dinv[nod[ok]]
        m["dvc"] = np.ascontiguousarray(dvl.reshape(NBLK, P).T)

        # masked/pad entries get slot 255: matches no iota value
        slot2 = np.full((P, max(ncol2, 1)), 255.0, np.float32)
        arp = np.arange(P, dtype=np.float32)
        for j, c3 in enumerate(cols2):
            if c3[0] == "diag":
                slot2[:, j] = arp
                continue
            i, b, ch = c3
            sl = slice(ch * P, (ch + 1) * P)
            bo, vv = blk_all[b]
            mask = (bo[sl] == i) & vv[sl]
            slot2[:, j] = np.where(mask, dl_all[b][sl], 255.0)
        # bake the one-hot S tiles [P, ncol2*P] fp8 (shared by L2 and L3;
        # diag cols come out as the identity via the same formula)
        import ml_dtypes as _mld
        S2 = (arp[None, None, :] == slot2[:, :, None])
        m["se2"] = np.ascontiguousarray(
            S2.reshape(P, -1).astype(_mld.float8_e4m3))
        dev.append(m)

    sched = dict(NPC=NPC, NBLK=NBLK, BR=BR, nch1=nch1, ncol1=ncol1,
                 parts1=parts1, emit1=emit1, emitQ1=emitQ1,
                 nch2=nch2, ncol2=ncol2, parts2=parts2, emit2=emit2,
                 emitS2=emitS2, diagcol=diagcol)
    return sched, dev


# ------------------------------------------------------------- device build
def _build(nc, tc, sched, N, D_IN, D_HID, D_OUT, N_CLS, bass, mybir):
    NPC, NBLK, BR = sched["NPC"], sched["NBLK"], sched["BR"]
    nch1, ncol1 = sched["nch1"], sched["ncol1"]
    parts1, emit1 = sched["parts1"], sched["emit1"]
    emitQ1 = sched["emitQ1"]
    nch2, ncol2 = sched["nch2"], sched["ncol2"]
    parts2, emit2 = sched["parts2"], sched["emit2"]
    emitS2, diagcol = sched["emitS2"], sched["diagcol"]
    f32 = mybir.dt.float32
    bf16 = mybir.dt.bfloat16
    AF = mybir.ActivationFunctionType
    OP = mybir.AluOpType
    RG = [list(range(NCORES))]
    lastP = NPC - (NBLK - 1) * P
    SP = not os.environ.get("GCN_NOSP")      # single_packet on gathers

    xe_d = nc.dram_tensor("xe", [P, nch1 * D_IN], bf16, kind="ExternalInput")
    W1_d = nc.dram_tensor("W1", [D_IN, D_HID], bf16, kind="ExternalInput")
    W2_d = nc.dram_tensor("W2", [D_HID, D_HID], bf16, kind="ExternalInput")
    W3_d = nc.dram_tensor("W3", [D_HID, D_OUT], bf16, kind="ExternalInput")
    Wo_d = nc.dram_tensor("Wout", [D_OUT, N_CLS], bf16, kind="ExternalInput")
    bn_d = {}
    for nm, dd in [("g1", D_HID), ("beta1", D_HID),
                   ("g2", D_HID), ("beta2", D_HID),
                   ("g3", D_OUT), ("beta3", D_OUT)]:
        bn_d[nm] = nc.dram_tensor(nm, [1, dd], f32, kind="ExternalInput")
    bo_d = nc.dram_tensor("bout", [N_CLS, 1], f32, kind="ExternalInput")
    iota4_d = nc.dram_tensor("iota4", [P, 4 * P], bf16, kind="ExternalInput")
    ones_d = nc.dram_tensor("ones", [P, 1], bf16, kind="ExternalInput")
    eye_d = nc.dram_tensor("eye", [P, P], bf16, kind="ExternalInput")
    eyef_d = nc.dram_tensor("eyef", [2, 2], f32, kind="ExternalInput")
    slot1_d = nc.dram_tensor("slot1", [P, ncol1], bf16, kind="ExternalInput")
    fp8 = mybir.dt.float8e4
    se2_d = nc.dram_tensor("se2", [P, max(ncol2, 1) * P], fp8,
                           kind="ExternalInput")
    dvc_d = nc.dram_tensor("dvc", [P, NBLK], f32, kind="ExternalInput")
    idx_d = [nc.dram_tensor(f"idx{b}", [P, max(nch2[b], 1) * 8], mybir.dt.int16,
                            kind="ExternalInput") for b in range(BANKS)]
    outT_d = nc.dram_tensor("outT", [N_CLS, NPC], f32, kind="ExternalOutput")

    pers = tc.alloc_tile_pool(name="pers", bufs=1)
    dram = tc.alloc_tile_pool(name="dram", bufs=1, space="DRAM")

    def ld(name, shape, dt_, src):
        t = pers.tile(shape, dt_, name=name)
        nc.sync.dma_start(out=t[:], in_=src)
        return t

    iota4 = ld("iota4", [P, 4 * P], bf16, iota4_d[:, :])
    dvc = ld("dvc", [P, NBLK], f32, dvc_d[:, :])
    ones = ld("ones", [P, 1], bf16, ones_d[:, :])
    eye = ld("eye", [P, P], bf16, eye_d[:, :])
    eyef = ld("eyef", [2, 2], f32, eyef_d[:, :])
    W1s = ld("W1s", [P, D_HID], bf16, W1_d[:, :])
    W2s = [ld(f"W2s{k}", [P, D_HID], bf16, W2_d[k * P:(k + 1) * P, :])
           for k in range(D_HID // P)]
    W3s = [ld(f"W3s{k}", [P, D_OUT], bf16, W3_d[k * P:(k + 1) * P, :])
           for k in range(D_HID // P)]
    Wos = ld("Wos", [P, N_CLS], bf16, Wo_d[:, :])
    bocol = ld("bocol", [N_CLS, 1], f32, bo_d[:, :])
    bnrow = {nm: ld(f"r_{nm}", [1, bn_d[nm].shape[1]], f32, bn_d[nm][:, :])
             for nm in bn_d}
    idx_s = [ld(f"idxs{b}", [P, max(nch2[b], 1) * 8], mybir.dt.int16,
                idx_d[b][:, :]) for b in range(BANKS)]

    ag1_in = dram.tile([NPC, D_HID], bf16, name="ag1_in")
    ag1_out = dram.tile([N, D_HID], bf16, name="ag1_out", addr_space="Shared")
    ag3_in = dram.tile([NPC, D_OUT], bf16, name="ag3_in")
    ag3_out = dram.tile([N, D_OUT], bf16, name="ag3_out", addr_space="Shared")

    stat = {}
    for li, dd in [(0, D_HID), (1, D_HID), (2, D_OUT)]:
        stat[li] = (pers.tile([1, dd], f32, name=f"ssum{li}"),
                    pers.tile([1, dd], f32, name=f"ssq{li}"))

    # --------------------------------------------------- shared inner pieces
    def conv_block(i, agg_ps, Wk, d_agg, d_out, csb, wp, ptr, pcv, sum_ps,
                   sq_ps, dscale=None):
        """agg psum -> (optional conv) -> csb[:, i*d_out:] + stats accum.
        dscale: per-partition dinv[dst] column folded out of the S tiles."""
        sc = dscale if dscale is not None else 1.0
        cslice = csb[:, i * d_out:(i + 1) * d_out]
        if Wk is None:
            nc.scalar.activation(out=cslice, in_=agg_ps[:], func=AF.Copy,
                                 scale=sc)
        else:
            agg_sb = wp.tile([P, d_agg], bf16, tag="aggsb")
            nc.scalar.activation(out=agg_sb[:], in_=agg_ps[:], func=AF.Copy,
                                 scale=sc)
            conv_ps = pcv.tile([P, d_out], f32, tag="conv")
            for k in range(d_agg // P):
                tp = ptr.tile([P, P], bf16, tag="tr")
                nc.tensor.transpose(out=tp[:], in_=agg_sb[:, k * P:(k + 1) * P],
                                    identity=eye[:])
                tsb = wp.tile([P, P], bf16, tag="aggT")
                nc.vector.tensor_copy(out=tsb[:], in_=tp[:])
                nc.tensor.matmul(out=conv_ps[:], lhsT=tsb[:], rhs=Wk[k][:],
                                 start=(k == 0), stop=(k == d_agg // P - 1))
            nc.scalar.activation(out=cslice, in_=conv_ps[:], func=AF.Copy)
        sq = wp.tile([P, d_out], bf16, tag="sq")
        nc.scalar.activation(out=sq[:], in_=cslice, func=AF.Square)
        nc.tensor.matmul(out=sum_ps[:], lhsT=ones[:], rhs=cslice,
                         start=(i == 0), stop=(i == NBLK - 1))
        nc.tensor.matmul(out=sq_ps[:], lhsT=ones[:], rhs=sq[:],
                         start=(i == 0), stop=(i == NBLK - 1))

    def bn_cols(li, gname, bename, d_out):
        stat_sum, stat_sq = stat[li]
        ar_in = dram.tile([2, d_out], f32, name=f"arin{li}")
        ar_out = dram.tile([2, d_out], f32, name=f"arout{li}", addr_space="Shared")
        nc.sync.dma_start(out=ar_in[0:1, :], in_=stat_sum[:])
        nc.sync.dma_start(out=ar_in[1:2, :], in_=stat_sq[:])
        nc.gpsimd.collective_compute(
            "AllReduce", OP.add, RG, ins=[ar_in[:, :]], outs=[ar_out[:, :]])
        with tc.tile_pool(name=f"bn{li}", bufs=1) as bp, \
             tc.tile_pool(name=f"bnp{li}", bufs=1, space="PSUM") as bpp:
            st0 = bp.tile([1, d_out], f32, name=f"st0{li}")
            nc.sync.dma_start(out=st0[:], in_=ar_out[0:1, :])
            st1 = bp.tile([1, d_out], f32, name=f"st1{li}")
            nc.sync.dma_start(out=st1[:], in_=ar_out[1:2, :])
            mean = bp.tile([1, d_out], f32, name=f"mean{li}")
            nc.vector.tensor_scalar(out=mean[:], in0=st0[:], scalar1=1.0 / N,
                                    scalar2=None, op0=OP.mult)
            ex2 = bp.tile([1, d_out], f32, name=f"ex2{li}")
            nc.vector.tensor_scalar(out=ex2[:], in0=st1[:], scalar1=1.0 / N,
                                    scalar2=None, op0=OP.mult)
            m2t = bp.tile([1, d_out], f32, name=f"m2{li}")
            nc.vector.tensor_tensor(out=m2t[:], in0=mean[:], in1=mean[:],
                                    op=OP.mult)
            var = bp.tile([1, d_out], f32, name=f"var{li}")
            nc.vector.tensor_tensor(out=var[:], in0=ex2[:], in1=m2t[:],
                                    op=OP.subtract)
            nc.vector.tensor_scalar(out=var[:], in0=var[:], scalar1=EPS,
                                    scalar2=None, op0=OP.add)
            sd = bp.tile([1, d_out], f32, name=f"sd{li}")
            nc.scalar.activation(out=sd[:], in_=var[:], func=AF.Sqrt)
            rinv = bp.tile([1, d_out], f32, name=f"rinv{li}")
            nc.vector.reciprocal(out=rinv[:], in_=sd[:])
            scl = bp.tile([1, d_out], f32, name=f"scl{li}")
            nc.vector.tensor_tensor(out=scl[:], in0=rinv[:],
                                    in1=bnrow[gname][:], op=OP.mult)
            tmp = bp.tile([1, d_out], f32, name=f"tmp{li}")
            nc.vector.tensor_tensor(out=tmp[:], in0=mean[:], in1=scl[:],
                                    op=OP.mult)
            sht = bp.tile([1, d_out], f32, name=f"sht{li}")
            nc.vector.tensor_tensor(out=sht[:], in0=bnrow[bename][:],
                                    in1=tmp[:], op=OP.subtract)
            pk = bp.tile([2, d_out], f32, name=f"pk{li}")
            nc.sync.dma_start(out=pk[0:1, :], in_=scl[:])
            nc.sync.dma_start(out=pk[1:2, :], in_=sht[:])
            cols = []
            for k in range(d_out // P):
                tp = bpp.tile([P, 2], f32, name=f"bnt{li}_{k}")
                nc.tensor.transpose(out=tp[:], in_=pk[:, k * P:(k + 1) * P],
                                    identity=eyef[:])
                cc = pers.tile([P, 2], f32, name=f"bncol{li}_{k}")
                nc.vector.tensor_copy(out=cc[:], in_=tp[:])
                cols.append(cc)
        return cols

    # ------------------------------------------------------------ L1 phase
    csb1_pool = tc.alloc_tile_pool(name="csb1p", bufs=1)
    csb1 = csb1_pool.tile([P, NBLK * D_HID], bf16, name="csb1")
    slot1_s = None
    with tc.tile_pool(name="l1t", bufs=1) as l1t, \
         tc.tile_pool(name="g1", bufs=3) as gp, \
         tc.tile_pool(name="w1p", bufs=3) as wp, \
         tc.tile_pool(name="s1p", bufs=4) as spool, \
         tc.tile_pool(name="ps1", bufs=2, space="PSUM") as pagg, \
         tc.tile_pool(name="pt1", bufs=2, space="PSUM") as ptr, \
         tc.tile_pool(name="pc1", bufs=2, space="PSUM") as pcv, \
         tc.tile_pool(name="pst1", bufs=1, space="PSUM") as pst:
        slot1_s = l1t.tile([P, ncol1], bf16, name="slot1s")
        nc.sync.dma_start(out=slot1_s[:], in_=slot1_d[:, :])
        sum_ps = pst.tile([1, D_HID], f32, name="sum1")
        sq_ps = pst.tile([1, D_HID], f32, name="sq1")
        piece = {}
        squad = {}
        for i in range(NBLK):
            for pi in emit1[i]:
                nchp = min(P_CH1, nch1 - pi * P_CH1)
                g = gp.tile([P, P_CH1 * D_IN], bf16, tag="gx", bufs=3,
                            name=f"gx{pi}")
                nc.sync.dma_start(
                    out=g[:, :nchp * D_IN],
                    in_=xe_d[:, pi * P_CH1 * D_IN:(pi * P_CH1 + nchp) * D_IN])
                piece[pi] = g
            for qi in emitQ1[i]:
                nq = min(4, ncol1 - qi * 4)
                S4 = spool.tile([P, 4 * P], fp8, tag="S4", bufs=6,
                                name=f"S4_{qi}")
                nc.vector.tensor_tensor(
                    out=S4[:, :nq * P].rearrange("p (c s) -> p c s", s=P),
                    in0=iota4[:, :nq * P].rearrange("p (c s) -> p c s", s=P),
                    in1=slot1_s[:, qi * 4:qi * 4 + nq].to_broadcast([P, nq, P]),
                    op=OP.is_equal)
                squad[qi] = S4
            agg_ps = pagg.tile([P, D_IN], f32, tag="agg")
            pl = parts1[i]
            for j, (ch, col) in enumerate(pl):
                qi, qc = divmod(col, 4)
                S_ap = squad[qi][:, qc * P:(qc + 1) * P]
                pi, c = divmod(ch, P_CH1)
                g = piece[pi]
                nc.tensor.matmul(
                    out=agg_ps[:], lhsT=S_ap,
                    rhs=g[:, c * D_IN:(c + 1) * D_IN],
                    start=(j == 0), stop=(j == len(pl) - 1))
            conv_block(i, agg_ps, [W1s], D_IN, D_HID, csb1, wp, ptr, pcv,
                       sum_ps, sq_ps)
        nc.vector.tensor_copy(out=stat[0][0][:], in_=sum_ps[:])
        nc.vector.tensor_copy(out=stat[0][1][:], in_=sq_ps[:])

    # ---------------- L1 apply -> ag1 (partial AllGathers overlap apply)
    c1 = bn_cols(0, "g1", "beta1", D_HID)
    AGQ1 = 4
    qb1 = [round(q * NBLK / AGQ1) for q in range(AGQ1 + 1)]
    ag1_or = ag1_out[:, :].rearrange("(c r) d -> c (r d)", c=NCORES)
    with tc.tile_pool(name="p3a", bufs=3) as wp, \
         tc.tile_pool(name="p3ap", bufs=4, space="PSUM") as pp:
        for i in range(NBLK):
            hrow = wp.tile([P, D_HID], bf16, tag="hrow")
            for k in range(D_HID // P):
                tp = pp.tile([P, P], bf16, tag="t1")
                nc.tensor.transpose(
                    out=tp[:],
                    in_=csb1[:, i * D_HID + k * P:i * D_HID + (k + 1) * P],
                    identity=eye[:])
                hT = wp.tile([P, P], bf16, tag="hT")
                nc.scalar.activation(out=hT[:], in_=tp[:], func=AF.Relu,
                                     scale=c1[k][:, 0:1], bias=c1[k][:, 1:2])
                tp2 = pp.tile([P, P], bf16, tag="t2")
                nc.tensor.transpose(out=tp2[:], in_=hT[:], identity=eye[:])
                nc.vector.tensor_scalar(
                    out=hrow[:, k * P:(k + 1) * P], in0=tp2[:],
                    scalar1=dvc[:, i:i + 1], scalar2=None, op0=OP.mult)
            rows = P if i < NBLK - 1 else lastP
            nc.sync.dma_start(out=ag1_in[i * P:i * P + rows, :],
                              in_=hrow[:rows, :])
    csb1_pool.release()
    nc.gpsimd.collective_compute(
        "AllGather", mybir.AluOpType.bypass, RG,
        ins=[ag1_in[:, :]], outs=[ag1_out[:, :]])

    # ------------------------------------------------- gather-based layer
    def gather_layer(table_ap, loc_ap, d_agg, Wk, d_out, csb, li):
        with tc.tile_pool(name=f"g{li}", bufs=3) as gp, \
             tc.tile_pool(name=f"w{li}", bufs=3) as wp, \
             tc.tile_pool(name=f"s{li}", bufs=4) as spool, \
             tc.tile_pool(name=f"ps{li}", bufs=2, space="PSUM") as pagg, \
             tc.tile_pool(name=f"pt{li}", bufs=2, space="PSUM") as ptr, \
             tc.tile_pool(name=f"pc{li}", bufs=2, space="PSUM") as pcv, \
             tc.tile_pool(name=f"pst{li}", bufs=1, space="PSUM") as pst:
            sum_ps = pst.tile([1, d_out], f32, name=f"sum{li}")
            sq_ps = pst.tile([1, d_out], f32, name=f"sq{li}")
            piece = {}
            spiece = {}
            for i in range(NBLK):
                for pi in emitS2[i]:
                    ncolp = min(SE_CH, ncol2 - pi * SE_CH)
                    se = spool.tile([P, SE_CH * P], fp8, tag="se", bufs=3,
                                    name=f"se{li}_{pi}")
                    nc.sync.dma_start(
                        out=se[:, :ncolp * P],
                        in_=se2_d[:, pi * SE_CH * P:(pi * SE_CH + ncolp) * P])
                    spiece[pi] = se
                for (b, pi) in emit2[i]:
                    cb0 = pi * P_CH
                    nchp = min(P_CH, nch2[b] - cb0)
                    if nchp <= 0:
                        continue
                    g = gp.tile([P, P_CH * d_agg], bf16, tag=f"gb{b}", bufs=3,
                                name=f"g{li}_{b}_{pi}")
                    nrows = min(BR, N - b * BR)
                    nc.gpsimd.dma_gather(
                        out_ap=g[:, :nchp * d_agg].rearrange(
                            "p (c d) -> p c d", d=d_agg),
                        in_ap=table_ap[b * BR:b * BR + nrows, :],
                        idxs_ap=idx_s[b][:, cb0 * 8:(cb0 + nchp) * 8],
                        num_idxs=nchp * P,
                        num_idxs_reg=nchp * P,
                        elem_size=d_agg,
                        single_packet=SP,
                        queue_num=b,
                    )
                    piece[(b, pi)] = g
                agg_ps = pagg.tile([P, d_agg], f32, tag="agg")
                # self-loop first: diag(dinv^2) @ local rows (no gather dep)
                dspi, dsc = divmod(diagcol[i], SE_CH)
                D_ap = spiece[dspi][:, dsc * P:(dsc + 1) * P]
                hloc = wp.tile([P, d_agg], bf16, tag="hloc")
                rows = P if i < NBLK - 1 else lastP
                if rows < P:
                    nc.vector.memset(hloc[:, :], 0.0)
                nc.sync.dma_start(out=hloc[:rows, :],
                                  in_=loc_ap[i * P:i * P + rows, :])
                nc.tensor.matmul(out=agg_ps[:], lhsT=D_ap, rhs=hloc[:],
                                 start=True, stop=False)
                pl = parts2[i]
                for j, (b, ch, col) in enumerate(pl):
                    spi, sc = divmod(col, SE_CH)
                    S_ap = spiece[spi][:, sc * P:(sc + 1) * P]
                    pi, c = divmod(ch, P_CH)
                    g = piece[(b, pi)]
                    nc.tensor.matmul(
                        out=agg_ps[:], lhsT=S_ap,
                        rhs=g[:, c * d_agg:(c + 1) * d_agg],
                        start=False, stop=(j == len(pl) - 1))
                conv_block(i, agg_ps, Wk, d_agg, d_out, csb, wp, ptr, pcv,
                           sum_ps, sq_ps, dscale=dvc[:, i:i + 1])
            nc.vector.tensor_copy(out=stat[li][0][:], in_=sum_ps[:])
            nc.vector.tensor_copy(out=stat[li][1][:], in_=sq_ps[:])

    # ---------------- layer 2
    csb2_pool = tc.alloc_tile_pool(name="csb2p", bufs=1)
    csb2 = csb2_pool.tile([P, NBLK * D_HID], bf16, name="csb2")
    gather_layer(ag1_out, ag1_in, D_HID, W2s, D_HID, csb2, 1)
    c2 = bn_cols(1, "g2", "beta2", D_HID)
    AGQ3 = 3
    qb3 = [round(q * NBLK / AGQ3) for q in range(AGQ3 + 1)]
    ag3_or = ag3_out[:, :].rearrange("(c r) d -> c (r d)", c=NCORES)
    with tc.tile_pool(name="p3b", bufs=3) as wp, \
         tc.tile_pool(name="p3bp", bufs=4, space="PSUM") as pp:
        for i in range(NBLK):
            p3 = pp.tile([P, D_OUT], f32, tag="p3")
            for k in range(D_HID // P):
                tp = pp.tile([P, P], bf16, tag="t1")
                nc.tensor.transpose(
                    out=tp[:],
                    in_=csb2[:, i * D_HID + k * P:i * D_HID + (k + 1) * P],
                    identity=eye[:])
                hT = wp.tile([P, P], bf16, tag="hT")
                nc.scalar.activation(out=hT[:], in_=tp[:], func=AF.Relu,
                                     scale=c2[k][:, 0:1], bias=c2[k][:, 1:2])
                nc.tensor.matmul(out=p3[:], lhsT=hT[:], rhs=W3s[k][:],
                                 start=(k == 0), stop=(k == D_HID // P - 1))
            c3sb = wp.tile([P, D_OUT], bf16, tag="c3")
            nc.vector.tensor_scalar(out=c3sb[:], in0=p3[:],
                                    scalar1=dvc[:, i:i + 1], scalar2=None,
                                    op0=OP.mult)
            rows = P if i < NBLK - 1 else lastP
            nc.sync.dma_start(out=ag3_in[i * P:i * P + rows, :],
                              in_=c3sb[:rows, :])
    csb2_pool.release()
    nc.gpsimd.collective_compute(
        "AllGather", mybir.AluOpType.bypass, RG,
        ins=[ag3_in[:, :]], outs=[ag3_out[:, :]])

    # ---------------- layer 3
    csb3_pool = tc.alloc_tile_pool(name="csb3p", bufs=1)
    csb3 = csb3_pool.tile([P, NBLK * D_OUT], bf16, name="csb3")
    gather_layer(ag3_out, ag3_in, D_OUT, None, D_OUT, csb3, 2)
    c3 = bn_cols(2, "g3", "beta3", D_OUT)
    with tc.tile_pool(name="p3c", bufs=3) as wp, \
         tc.tile_pool(name="p3cp", bufs=4, space="PSUM") as pp:
        for i in range(NBLK):
            tp = pp.tile([P, P], bf16, tag="t1")
            nc.tensor.transpose(out=tp[:], in_=csb3[:, i * P:(i + 1) * P],
                                identity=eye[:])
            xT = wp.tile([P, P], bf16, tag="xT")
            nc.scalar.activation(out=xT[:], in_=tp[:], func=AF.Relu,
                                 scale=c3[0][:, 0:1], bias=c3[0][:, 1:2])
            po = pp.tile([N_CLS, P], f32, tag="po")
            nc.tensor.matmul(out=po[:], lhsT=Wos[:], rhs=xT[:],
                             start=True, stop=True)
            osb = wp.tile([N_CLS, P], f32, tag="osb")
            nc.vector.tensor_scalar(out=osb[:], in0=po[:],
                                    scalar1=bocol[:, 0:1], scalar2=None,
                                    op0=OP.add)
            rows = P if i < NBLK - 1 else lastP
            nc.sync.dma_start(out=outT_d[:, i * P:i * P + rows],
                              in_=osb[:, :rows])
    csb3_pool.release()
    pers.release()
    dram.release()


# ------------------------------------------------------------------ kernel
def kernel(x, edge_index, W1, b1, g1, beta1, W2, b2, g2, beta2,
           W3, b3, g3, beta3, Wout, bout):
    import ml_dtypes
    import concourse.bass as bass
    import concourse.mybir as mybir
    from concourse.bacc import Bacc
    from concourse.tile import TileContext
    from concourse.bass_utils import run_bass_kernel_spmd

    x = np.asarray(x, dtype=np.float32)
    ei = np.asarray(edge_index, dtype=np.int64)
    N, D_IN = x.shape
    D_HID = np.asarray(W1).shape[1]
    D_OUT = np.asarray(W3).shape[1]
    N_CLS = np.asarray(Wout).shape[1]
    bf = ml_dtypes.bfloat16

    sched, dev = _host_prep(N, ei[0], ei[1], x, bf)

    nc = Bacc(num_devices=NCORES, num_swdge_queues=4)
    with TileContext(nc) as tc:
        _build(nc, tc, sched, N, D_IN, D_HID, D_OUT, N_CLS, bass, mybir)
    nc.compile()

    common = {
        "W1": np.asarray(W1, np.float32).astype(bf),
        "W2": np.asarray(W2, np.float32).astype(bf),
        "W3": np.asarray(W3, np.float32).astype(bf),
        "Wout": np.asarray(Wout, np.float32).astype(bf),
        "g1": np.asarray(g1, np.float32).reshape(1, -1),
        "beta1": np.asarray(beta1, np.float32).reshape(1, -1),
        "g2": np.asarray(g2, np.float32).reshape(1, -1),
        "beta2": np.asarray(beta2, np.float32).reshape(1, -1),
        "g3": np.asarray(g3, np.float32).reshape(1, -1),
        "beta3": np.asarray(beta3, np.float32).reshape(1, -1),
        "bout": np.asarray(bout, np.float32).reshape(-1, 1),
        "iota4": np.tile(np.arange(P, dtype=np.float32), (P, 4)).astype(bf),
        "ones": np.ones((P, 1), np.float32).astype(bf),
        "eye": np.eye(P, dtype=np.float32).astype(bf),
        "eyef": np.eye(2, dtype=np.float32),
    }
    in_maps = [{**common, **dev[c]} for c in range(NCORES)]
    _trace = bool(os.environ.get("GCN_TRACE"))
    res = run_bass_kernel_spmd(nc, in_maps, core_ids=list(range(NCORES)),
                               trace=_trace)
    out = np.concatenate([res.results[c]["outT"].T for c in range(NCORES)], axis=0)
    kernel._last_res = res
    return out


# revision 60
# speedup vs baseline: 1.0251x; 1.0251x over previous
"""GCN (3x GCNConv+BN+ReLU, linear head) on 8 TRN2 NeuronCores.

Strategy (graph/data parallel, dst-partitioned, bf16 data path):
  - Nodes row-sharded 8 ways; edges partitioned by dst core, grouped by
    dst block (128 nodes), per-block counts aligned across cores (~3% pad)
    so the SPMD program structure is identical on every core.
  - L1: NO device gathers. Host pre-builds an edge-major bf16 feature
    image (already in SBUF layout) streamed sequentially at full DMA
    bandwidth.
  - L2/L3: SWDGE dma_gather (4 queues, bf16 elems) of halo rows from the
    all-gathered activations; self-loops excluded from the gather stream
    and applied per block via a diagonal one-hot matmul reading the
    core-local rows with a sequential DMA.
  - Per 128-edge chunk: one-hot S (iota-compare, DVE, bf16) and a
    segment-sum matmul accumulating in PSUM per 128-node block. Chunks
    are densely packed; a chunk straddling two blocks is applied twice
    with disjoint norm masks.
  - BatchNorm: per-feature sums via ones-matmul + tiny AllReduce; conv
    outputs stay resident in SBUF; affine+ReLU applied in transposed
    layout as one ACT op per tile. Conv bias cancels in train-mode BN.
  - Activations all-gathered between layers in bf16.
"""
import os

import numpy as np

P = 128
NCORES = 8
BANKS = 4
P_CH = 8          # chunks per dma_gather piece
P_CH1 = 32        # chunks per L1 stream-load piece
SE_CH = 32        # S-tile cols per stream-load piece (L2/L3)
EPS = 1e-5


def _pack_idx(idx_flat):
    # [n] int16 -> [128, n//16] wrapped in 16 partitions, replicated x8
    return np.tile(idx_flat.reshape(-1, 16).T, (8, 1)).copy()


# ----------------------------------------------------------------- host prep
def _host_prep(N, src, dst, x32, bf):
    """Build per-core streams/tables + shared static schedule."""
    NPC = N // NCORES
    NBLK = (NPC + P - 1) // P
    BR = -(-N // BANKS)
    D_IN = x32.shape[1]

    loops = np.arange(N, dtype=np.int64)
    src_f = np.concatenate([src, loops])
    dst_f = np.concatenate([dst, loops])
    deg = np.bincount(dst_f, minlength=N).astype(np.float32)
    dinv = np.where(deg > 0, 1.0 / np.sqrt(deg), 0.0).astype(np.float32)
    w_f = (dinv[src_f] * dinv[dst_f]).astype(np.float32)

    # ---------- L1 stream: all edges incl self-loops, grouped by dst block
    core1 = dst_f // NPC
    per1 = []
    for c in range(NCORES):
        sel = np.flatnonzero(core1 == c)
        s, d, ww = src_f[sel], dst_f[sel] - c * NPC, w_f[sel]
        blk = d // P
        o = np.argsort(blk, kind="stable")
        per1.append((s[o], d[o], ww[o], blk[o]))
    cnt1 = np.zeros((NCORES, NBLK), np.int64)
    for c in range(NCORES):
        np.add.at(cnt1[c], per1[c][3], 1)
    m1 = cnt1.max(axis=0)                      # aligned per-block counts
    ofs1 = np.cumsum(m1) - m1
    L1 = int(m1.sum())
    nch1 = -(-L1 // P)
    L1p = nch1 * P

    cols1 = []                                  # (block, chunk)
    parts1 = [[] for _ in range(NBLK)]          # block -> [(chunk, col)]
    for i in range(NBLK):
        if m1[i] == 0:
            continue
        c0 = int(ofs1[i]) // P
        c1 = int(ofs1[i] + m1[i] - 1) // P
        for ch in range(c0, c1 + 1):
            parts1[i].append((ch, len(cols1)))
            cols1.append((i, ch))
    ncol1 = len(cols1)

    # ---------- L2/L3 streams: real edges only, by (dst block, src bank)
    core2 = dst // NPC
    w_r = w_f[: len(src)]
    per2 = []
    for c in range(NCORES):
        sel = np.flatnonzero(core2 == c)
        s, d, ww = src[sel], dst[sel] - c * NPC, w_r[sel]
        blk = d // P
        bank = s // BR
        o = np.argsort(blk * BANKS + bank, kind="stable")
        per2.append((s[o], d[o], ww[o], blk[o], bank[o]))
    cnt2 = np.zeros((NCORES, NBLK, BANKS), np.int64)
    for c in range(NCORES):
        np.add.at(cnt2[c], (per2[c][3], per2[c][4]), 1)
    m2 = cnt2.max(axis=0)                       # [NBLK, BANKS]
    ofs2 = np.cumsum(m2, axis=0) - m2           # stream offset per (blk,bank)
    L2 = m2.sum(axis=0)                         # [BANKS] stream lengths
    nch2 = [int(-(-max(int(L2[b]), 1) // P)) for b in range(BANKS)]

    cols2 = []                                  # (block, bank, chunk) | diag
    parts2 = [[] for _ in range(NBLK)]          # block -> [(bank, chunk, col)]
    diagcol = [0] * NBLK                        # block -> col of its diag tile
    for i in range(NBLK):
        diagcol[i] = len(cols2)
        cols2.append(("diag", i, -1))
        for b in range(BANKS):
            if m2[i, b] == 0:
                continue
            c0 = int(ofs2[i, b]) // P
            c1 = int(ofs2[i, b] + m2[i, b] - 1) // P
            for ch in range(c0, c1 + 1):
                parts2[i].append((b, ch, len(cols2)))
                cols2.append((i, b, ch))
    ncol2 = len(cols2)

    # piece emission schedule (stream pieces to load before block i)
    emit1 = [[] for _ in range(NBLK)]
    seen = set()
    for i in range(NBLK):
        for (ch, _) in parts1[i]:
            pi = ch // P_CH1
            if pi not in seen:
                seen.add(pi)
                emit1[i].append(pi)
    emitQ1 = [[] for _ in range(NBLK)]          # quad one-hot build schedule
    seen = set()
    for i in range(NBLK):
        for (_, col) in parts1[i]:
            qi = col // 4
            if qi not in seen:
                seen.add(qi)
                emitQ1[i].append(qi)
    emit2 = [[] for _ in range(NBLK)]
    seen = set()
    for i in range(NBLK):
        for (b, ch, _) in parts2[i]:
            pi = ch // P_CH
            if (b, pi) not in seen:
                seen.add((b, pi))
                emit2[i].append((b, pi))
    emitS2 = [[] for _ in range(NBLK)]
    seen = set()
    for i in range(NBLK):
        for col in [diagcol[i]] + [c for (_, _, c) in parts2[i]]:
            pi = col // SE_CH
            if pi not in seen:
                seen.add(pi)
                emitS2[i].append(pi)

    # ---------- per-core device arrays
    dev = []
    for c in range(NCORES):
        m = {}
        # L1: edge-major bf16 image [128, nch1*D_IN] + slot/norm cols
        s, d, ww, blk = per1[c]
        pos_in = np.zeros(L1p, np.int64)        # stream pos -> edge idx+1
        cur = 0
        for i in range(NBLK):
            n = int(cnt1[c][i])
            pos_in[int(ofs1[i]):int(ofs1[i]) + n] = np.arange(cur, cur + n) + 1
            cur += n
        srcs = np.zeros(L1p, np.int64)
        valid = pos_in > 0
        srcs[valid] = s[pos_in[valid] - 1]
        dloc = np.zeros(L1p, np.float32)
        wloc = np.zeros(L1p, np.float32)
        dloc[valid] = (d[pos_in[valid] - 1] % P).astype(np.float32)
        wloc[valid] = ww[pos_in[valid] - 1]
        # fold the edge norm into the streamed rows; S becomes a pure one-hot
        xe = (x32[srcs] * wloc[:, None]).astype(bf)   # [L1p, D_IN]
        m["xe"] = np.ascontiguousarray(
            xe.reshape(nch1, P, D_IN).transpose(1, 0, 2).reshape(P, nch1 * D_IN))
        blk_of = np.full(L1p, -1, np.int64)
        for i in range(NBLK):
            blk_of[int(ofs1[i]):int(ofs1[i] + m1[i])] = i
        # masked/pad entries get slot 255: matches no iota value
        slot1 = np.full((P, ncol1), 255.0, np.float32)
        for j, (i, ch) in enumerate(cols1):
            sl = slice(ch * P, (ch + 1) * P)
            mask = (blk_of[sl] == i) & valid[sl]
            slot1[:, j] = np.where(mask, dloc[sl], 255.0)
        m["slot1"] = slot1.astype(bf)

        # L2/L3: idx banks + slot/norm cols
        s, d, ww, blk, bank = per2[c]
        dl_all, w_all, blk_all = [], [], []
        for b in range(BANKS):
            Lb = nch2[b] * P
            idx = np.zeros(Lb, np.int16)
            dl = np.zeros(Lb, np.float32)
            wl = np.zeros(Lb, np.float32)
            bo = np.full(Lb, -1, np.int64)
            vv = np.zeros(Lb, bool)
            selb = bank == b
            sb, db, wb = s[selb], d[selb], ww[selb]
            cpos = 0
            for i in range(NBLK):
                n = int(cnt2[c][i, b])
                o0 = int(ofs2[i, b])
                idx[o0:o0 + n] = (sb[cpos:cpos + n] - b * BR).astype(np.int16)
                dl[o0:o0 + n] = (db[cpos:cpos + n] % P).astype(np.float32)
                wl[o0:o0 + n] = wb[cpos:cpos + n]
                vv[o0:o0 + n] = True
                bo[o0:o0 + int(m2[i, b])] = i
                cpos += n
            dl_all.append(dl)
            w_all.append(wl)
            blk_all.append((bo, vv))
            m[f"idx{b}"] = _pack_idx(idx)
        # per-node dinv columns [128, NBLK]: dinv[src] is folded into the
        # apply-phase row writes, dinv[dst] into the aggregate-copy scale,
        # so the S tiles are PURE one-hots (exactly representable in fp8).
        nod = c * NPC + np.arange(NBLK * P)
        dvl = np.zeros(NBLK * P, np.float32)
        ok = nod < (c + 1) * NPC
        dvl[ok] = dinv[nod[ok]]
        m["dvc"] = np.ascontiguousarray(dvl.reshape(NBLK, P).T)

        # masked/pad entries get slot 255: matches no iota value
        slot2 = np.full((P, max(ncol2, 1)), 255.0, np.float32)
        arp = np.arange(P, dtype=np.float32)
        for j, c3 in enumerate(cols2):
            if c3[0] == "diag":
                slot2[:, j] = arp
                continue
            i, b, ch = c3
            sl = slice(ch * P, (ch + 1) * P)
            bo, vv = blk_all[b]
            mask = (bo[sl] == i) & vv[sl]
            slot2[:, j] = np.where(mask, dl_all[b][sl], 255.0)
        # bake the one-hot S tiles [P, ncol2*P] fp8 (shared by L2 and L3;
        # diag cols come out as the identity via the same formula)
        import ml_dtypes as _mld
        S2 = (arp[None, None, :] == slot2[:, :, None])
        m["se2"] = np.ascontiguousarray(
            S2.reshape(P, -1).astype(_mld.float8_e4m3))
        dev.append(m)

    sched = dict(NPC=NPC, NBLK=NBLK, BR=BR, nch1=nch1, ncol1=ncol1,
                 parts1=parts1, emit1=emit1, emitQ1=emitQ1,
                 nch2=nch2, ncol2=ncol2, parts2=parts2, emit2=emit2,
                 emitS2=emitS2, diagcol=diagcol)
    return sched, dev


# ------------------------------------------------------------- device build
def _build(nc, tc, sched, N, D_IN, D_HID, D_OUT, N_CLS, bass, mybir):
    NPC, NBLK, BR = sched["NPC"], sched["NBLK"], sched["BR"]
    nch1, ncol1 = sched["nch1"], sched["ncol1"]
    parts1, emit1 = sched["parts1"], sched["emit1"]
    emitQ1 = sched["emitQ1"]
    nch2, ncol2 = sched["nch2"], sched["ncol2"]
    parts2, emit2 = sched["parts2"], sched["emit2"]
    emitS2, diagcol = sched["emitS2"], sched["diagcol"]
    f32 = mybir.dt.float32
    bf16 = mybir.dt.bfloat16
    AF = mybir.ActivationFunctionType
    OP = mybir.AluOpType
    RG = [list(range(NCORES))]
    lastP = NPC - (NBLK - 1) * P
    SP = not os.environ.get("GCN_NOSP")      # single_packet on gathers

    xe_d = nc.dram_tensor("xe", [P, nch1 * D_IN], bf16, kind="ExternalInput")
    W1_d = nc.dram_tensor("W1", [D_IN, D_HID], bf16, kind="ExternalInput")
    W2_d = nc.dram_tensor("W2", [D_HID, D_HID], bf16, kind="ExternalInput")
    W3_d = nc.dram_tensor("W3", [D_HID, D_OUT], bf16, kind="ExternalInput")
    Wo_d = nc.dram_tensor("Wout", [D_OUT, N_CLS], bf16, kind="ExternalInput")
    bn_d = {}
    for nm, dd in [("g1", D_HID), ("beta1", D_HID),
                   ("g2", D_HID), ("beta2", D_HID),
                   ("g3", D_OUT), ("beta3", D_OUT)]:
        bn_d[nm] = nc.dram_tensor(nm, [1, dd], f32, kind="ExternalInput")
    bo_d = nc.dram_tensor("bout", [N_CLS, 1], f32, kind="ExternalInput")
    iota4_d = nc.dram_tensor("iota4", [P, 4 * P], bf16, kind="ExternalInput")
    ones_d = nc.dram_tensor("ones", [P, 1], bf16, kind="ExternalInput")
    eye_d = nc.dram_tensor("eye", [P, P], bf16, kind="ExternalInput")
    eyef_d = nc.dram_tensor("eyef", [2, 2], f32, kind="ExternalInput")
    slot1_d = nc.dram_tensor("slot1", [P, ncol1], bf16, kind="ExternalInput")
    fp8 = mybir.dt.float8e4
    se2_d = nc.dram_tensor("se2", [P, max(ncol2, 1) * P], fp8,
                           kind="ExternalInput")
    dvc_d = nc.dram_tensor("dvc", [P, NBLK], f32, kind="ExternalInput")
    idx_d = [nc.dram_tensor(f"idx{b}", [P, max(nch2[b], 1) * 8], mybir.dt.int16,
                            kind="ExternalInput") for b in range(BANKS)]
    outT_d = nc.dram_tensor("outT", [N_CLS, NPC], f32, kind="ExternalOutput")

    pers = tc.alloc_tile_pool(name="pers", bufs=1)
    dram = tc.alloc_tile_pool(name="dram", bufs=1, space="DRAM")

    def ld(name, shape, dt_, src):
        t = pers.tile(shape, dt_, name=name)
        nc.sync.dma_start(out=t[:], in_=src)
        return t

    iota4 = ld("iota4", [P, 4 * P], bf16, iota4_d[:, :])
    dvc = ld("dvc", [P, NBLK], f32, dvc_d[:, :])
    ones = ld("ones", [P, 1], bf16, ones_d[:, :])
    eye = ld("eye", [P, P], bf16, eye_d[:, :])
    eyef = ld("eyef", [2, 2], f32, eyef_d[:, :])
    W1s = ld("W1s", [P, D_HID], bf16, W1_d[:, :])
    W2s = [ld(f"W2s{k}", [P, D_HID], bf16, W2_d[k * P:(k + 1) * P, :])
           for k in range(D_HID // P)]
    W3s = [ld(f"W3s{k}", [P, D_OUT], bf16, W3_d[k * P:(k + 1) * P, :])
           for k in range(D_HID // P)]
    Wos = ld("Wos", [P, N_CLS], bf16, Wo_d[:, :])
    bocol = ld("bocol", [N_CLS, 1], f32, bo_d[:, :])
    bnrow = {nm: ld(f"r_{nm}", [1, bn_d[nm].shape[1]], f32, bn_d[nm][:, :])
             for nm in bn_d}
    idx_s = [ld(f"idxs{b}", [P, max(nch2[b], 1) * 8], mybir.dt.int16,
                idx_d[b][:, :]) for b in range(BANKS)]

    ag1_in = dram.tile([NPC, D_HID], bf16, name="ag1_in")
    ag1_out = dram.tile([N, D_HID], bf16, name="ag1_out", addr_space="Shared")
    ag3_in = dram.tile([NPC, D_OUT], bf16, name="ag3_in")
    ag3_out = dram.tile([N, D_OUT], bf16, name="ag3_out", addr_space="Shared")

    stat = {}
    for li, dd in [(0, D_HID), (1, D_HID), (2, D_OUT)]:
        stat[li] = (pers.tile([1, dd], f32, name=f"ssum{li}"),
                    pers.tile([1, dd], f32, name=f"ssq{li}"))

    # --------------------------------------------------- shared inner pieces
    def conv_block(i, agg_ps, Wk, d_agg, d_out, csb, wp, ptr, pcv, sum_ps,
                   sq_ps, dscale=None):
        """agg psum -> (optional conv) -> csb[:, i*d_out:] + stats accum.
        dscale: per-partition dinv[dst] column folded out of the S tiles."""
        sc = dscale if dscale is not None else 1.0
        cslice = csb[:, i * d_out:(i + 1) * d_out]
        if Wk is None:
            nc.scalar.activation(out=cslice, in_=agg_ps[:], func=AF.Copy,
                                 scale=sc)
        else:
            agg_sb = wp.tile([P, d_agg], bf16, tag="aggsb")
            nc.scalar.activation(out=agg_sb[:], in_=agg_ps[:], func=AF.Copy,
                                 scale=sc)
            conv_ps = pcv.tile([P, d_out], f32, tag="conv")
            for k in range(d_agg // P):
                tp = ptr.tile([P, P], bf16, tag="tr")
                nc.tensor.transpose(out=tp[:], in_=agg_sb[:, k * P:(k + 1) * P],
                                    identity=eye[:])
                tsb = wp.tile([P, P], bf16, tag="aggT")
                nc.vector.tensor_copy(out=tsb[:], in_=tp[:])
                nc.tensor.matmul(out=conv_ps[:], lhsT=tsb[:], rhs=Wk[k][:],
                                 start=(k == 0), stop=(k == d_agg // P - 1))
            nc.scalar.activation(out=cslice, in_=conv_ps[:], func=AF.Copy)
        sq = wp.tile([P, d_out], bf16, tag="sq")
        nc.scalar.activation(out=sq[:], in_=cslice, func=AF.Square)
        nc.tensor.matmul(out=sum_ps[:], lhsT=ones[:], rhs=cslice,
                         start=(i == 0), stop=(i == NBLK - 1))
        nc.tensor.matmul(out=sq_ps[:], lhsT=ones[:], rhs=sq[:],
                         start=(i == 0), stop=(i == NBLK - 1))

    def bn_cols(li, gname, bename, d_out):
        stat_sum, stat_sq = stat[li]
        ar_in = dram.tile([2, d_out], f32, name=f"arin{li}")
        ar_out = dram.tile([2, d_out], f32, name=f"arout{li}", addr_space="Shared")
        nc.sync.dma_start(out=ar_in[0:1, :], in_=stat_sum[:])
        nc.sync.dma_start(out=ar_in[1:2, :], in_=stat_sq[:])
        nc.gpsimd.collective_compute(
            "AllReduce", OP.add, RG, ins=[ar_in[:, :]], outs=[ar_out[:, :]])
        with tc.tile_pool(name=f"bn{li}", bufs=1) as bp, \
             tc.tile_pool(name=f"bnp{li}", bufs=1, space="PSUM") as bpp:
            st0 = bp.tile([1, d_out], f32, name=f"st0{li}")
            nc.sync.dma_start(out=st0[:], in_=ar_out[0:1, :])
            st1 = bp.tile([1, d_out], f32, name=f"st1{li}")
            nc.sync.dma_start(out=st1[:], in_=ar_out[1:2, :])
            mean = bp.tile([1, d_out], f32, name=f"mean{li}")
            nc.vector.tensor_scalar(out=mean[:], in0=st0[:], scalar1=1.0 / N,
                                    scalar2=None, op0=OP.mult)
            ex2 = bp.tile([1, d_out], f32, name=f"ex2{li}")
            nc.vector.tensor_scalar(out=ex2[:], in0=st1[:], scalar1=1.0 / N,
                                    scalar2=None, op0=OP.mult)
            m2t = bp.tile([1, d_out], f32, name=f"m2{li}")
            nc.vector.tensor_tensor(out=m2t[:], in0=mean[:], in1=mean[:],
                                    op=OP.mult)
            var = bp.tile([1, d_out], f32, name=f"var{li}")
            nc.vector.tensor_tensor(out=var[:], in0=ex2[:], in1=m2t[:],
                                    op=OP.subtract)
            nc.vector.tensor_scalar(out=var[:], in0=var[:], scalar1=EPS,
                                    scalar2=None, op0=OP.add)
            sd = bp.tile([1, d_out], f32, name=f"sd{li}")
            nc.scalar.activation(out=sd[:], in_=var[:], func=AF.Sqrt)
            rinv = bp.tile([1, d_out], f32, name=f"rinv{li}")
            nc.vector.reciprocal(out=rinv[:], in_=sd[:])
            scl = bp.tile([1, d_out], f32, name=f"scl{li}")
            nc.vector.tensor_tensor(out=scl[:], in0=rinv[:],
                                    in1=bnrow[gname][:], op=OP.mult)
            tmp = bp.tile([1, d_out], f32, name=f"tmp{li}")
            nc.vector.tensor_tensor(out=tmp[:], in0=mean[:], in1=scl[:],
                                    op=OP.mult)
            sht = bp.tile([1, d_out], f32, name=f"sht{li}")
            nc.vector.tensor_tensor(out=sht[:], in0=bnrow[bename][:],
                                    in1=tmp[:], op=OP.subtract)
            pk = bp.tile([2, d_out], f32, name=f"pk{li}")
            nc.sync.dma_start(out=pk[0:1, :], in_=scl[:])
            nc.sync.dma_start(out=pk[1:2, :], in_=sht[:])
            cols = []
            for k in range(d_out // P):
                tp = bpp.tile([P, 2], f32, name=f"bnt{li}_{k}")
                nc.tensor.transpose(out=tp[:], in_=pk[:, k * P:(k + 1) * P],
                                    identity=eyef[:])
                cc = pers.tile([P, 2], f32, name=f"bncol{li}_{k}")
                nc.vector.tensor_copy(out=cc[:], in_=tp[:])
                cols.append(cc)
        return cols

    # ------------------------------------------------------------ L1 phase
    csb1_pool = tc.alloc_tile_pool(name="csb1p", bufs=1)
    csb1 = csb1_pool.tile([P, NBLK * D_HID], bf16, name="csb1")
    slot1_s = None
    with tc.tile_pool(name="l1t", bufs=1) as l1t, \
         tc.tile_pool(name="g1", bufs=3) as gp, \
         tc.tile_pool(name="w1p", bufs=3) as wp, \
         tc.tile_pool(name="s1p", bufs=4) as spool, \
         tc.tile_pool(name="ps1", bufs=2, space="PSUM") as pagg, \
         tc.tile_pool(name="pt1", bufs=2, space="PSUM") as ptr, \
         tc.tile_pool(name="pc1", bufs=2, space="PSUM") as pcv, \
         tc.tile_pool(name="pst1", bufs=1, space="PSUM") as pst:
        slot1_s = l1t.tile([P, ncol1], bf16, name="slot1s")
        nc.sync.dma_start(out=slot1_s[:], in_=slot1_d[:, :])
        sum_ps = pst.tile([1, D_HID], f32, name="sum1")
        sq_ps = pst.tile([1, D_HID], f32, name="sq1")
        piece = {}
        squad = {}
        for i in range(NBLK):
            for pi in emit1[i]:
                nchp = min(P_CH1, nch1 - pi * P_CH1)
                g = gp.tile([P, P_CH1 * D_IN], bf16, tag="gx", bufs=3,
                            name=f"gx{pi}")
                nc.sync.dma_start(
                    out=g[:, :nchp * D_IN],
                    in_=xe_d[:, pi * P_CH1 * D_IN:(pi * P_CH1 + nchp) * D_IN])
                piece[pi] = g
            for qi in emitQ1[i]:
                nq = min(4, ncol1 - qi * 4)
                S4 = spool.tile([P, 4 * P], bf16, tag="S4", bufs=6,
                                name=f"S4_{qi}")
                nc.vector.tensor_tensor(
                    out=S4[:, :nq * P].rearrange("p (c s) -> p c s", s=P),
                    in0=iota4[:, :nq * P].rearrange("p (c s) -> p c s", s=P),
                    in1=slot1_s[:, qi * 4:qi * 4 + nq].to_broadcast([P, nq, P]),
                    op=OP.is_equal)
                squad[qi] = S4
            agg_ps = pagg.tile([P, D_IN], f32, tag="agg")
            pl = parts1[i]
            for j, (ch, col) in enumerate(pl):
                qi, qc = divmod(col, 4)
                S_ap = squad[qi][:, qc * P:(qc + 1) * P]
                pi, c = divmod(ch, P_CH1)
                g = piece[pi]
                nc.tensor.matmul(
                    out=agg_ps[:], lhsT=S_ap,
                    rhs=g[:, c * D_IN:(c + 1) * D_IN],
                    start=(j == 0), stop=(j == len(pl) - 1))
            conv_block(i, agg_ps, [W1s], D_IN, D_HID, csb1, wp, ptr, pcv,
                       sum_ps, sq_ps)
        nc.vector.tensor_copy(out=stat[0][0][:], in_=sum_ps[:])
        nc.vector.tensor_copy(out=stat[0][1][:], in_=sq_ps[:])

    # ---------------- L1 apply -> ag1 (partial AllGathers overlap apply)
    c1 = bn_cols(0, "g1", "beta1", D_HID)
    AGQ1 = 4
    qb1 = [round(q * NBLK / AGQ1) for q in range(AGQ1 + 1)]
    ag1_or = ag1_out[:, :].rearrange("(c r) d -> c (r d)", c=NCORES)
    with tc.tile_pool(name="p3a", bufs=3) as wp, \
         tc.tile_pool(name="p3ap", bufs=4, space="PSUM") as pp:
        for i in range(NBLK):
            hrow = wp.tile([P, D_HID], bf16, tag="hrow")
            for k in range(D_HID // P):
                tp = pp.tile([P, P], bf16, tag="t1")
                nc.tensor.transpose(
                    out=tp[:],
                    in_=csb1[:, i * D_HID + k * P:i * D_HID + (k + 1) * P],
                    identity=eye[:])
                hT = wp.tile([P, P], bf16, tag="hT")
                nc.scalar.activation(out=hT[:], in_=tp[:], func=AF.Relu,
                                     scale=c1[k][:, 0:1], bias=c1[k][:, 1:2])
                tp2 = pp.tile([P, P], bf16, tag="t2")
                nc.tensor.transpose(out=tp2[:], in_=hT[:], identity=eye[:])
                nc.vector.tensor_scalar(
                    out=hrow[:, k * P:(k + 1) * P], in0=tp2[:],
                    scalar1=dvc[:, i:i + 1], scalar2=None, op0=OP.mult)
            rows = P if i < NBLK - 1 else lastP
            nc.sync.dma_start(out=ag1_in[i * P:i * P + rows, :],
                              in_=hrow[:rows, :])
    csb1_pool.release()
    nc.gpsimd.collective_compute(
        "AllGather", mybir.AluOpType.bypass, RG,
        ins=[ag1_in[:, :]], outs=[ag1_out[:, :]])

    # ------------------------------------------------- gather-based layer
    def gather_layer(table_ap, loc_ap, d_agg, Wk, d_out, csb, li):
        with tc.tile_pool(name=f"g{li}", bufs=3) as gp, \
             tc.tile_pool(name=f"w{li}", bufs=3) as wp, \
             tc.tile_pool(name=f"s{li}", bufs=4) as spool, \
             tc.tile_pool(name=f"ps{li}", bufs=2, space="PSUM") as pagg, \
             tc.tile_pool(name=f"pt{li}", bufs=2, space="PSUM") as ptr, \
             tc.tile_pool(name=f"pc{li}", bufs=2, space="PSUM") as pcv, \
             tc.tile_pool(name=f"pst{li}", bufs=1, space="PSUM") as pst:
            sum_ps = pst.tile([1, d_out], f32, name=f"sum{li}")
            sq_ps = pst.tile([1, d_out], f32, name=f"sq{li}")
            piece = {}
            spiece = {}
            for i in range(NBLK):
                for pi in emitS2[i]:
                    ncolp = min(SE_CH, ncol2 - pi * SE_CH)
                    se = spool.tile([P, SE_CH * P], fp8, tag="se", bufs=3,
                                    name=f"se{li}_{pi}")
                    nc.sync.dma_start(
                        out=se[:, :ncolp * P],
                        in_=se2_d[:, pi * SE_CH * P:(pi * SE_CH + ncolp) * P])
                    spiece[pi] = se
                for (b, pi) in emit2[i]:
                    cb0 = pi * P_CH
                    nchp = min(P_CH, nch2[b] - cb0)
                    if nchp <= 0:
                        continue
                    g = gp.tile([P, P_CH * d_agg], bf16, tag=f"gb{b}", bufs=3,
                                name=f"g{li}_{b}_{pi}")
                    nrows = min(BR, N - b * BR)
                    nc.gpsimd.dma_gather(
                        out_ap=g[:, :nchp * d_agg].rearrange(
                            "p (c d) -> p c d", d=d_agg),
                        in_ap=table_ap[b * BR:b * BR + nrows, :],
                        idxs_ap=idx_s[b][:, cb0 * 8:(cb0 + nchp) * 8],
                        num_idxs=nchp * P,
                        num_idxs_reg=nchp * P,
                        elem_size=d_agg,
                        single_packet=SP,
                        queue_num=b,
                    )
                    piece[(b, pi)] = g
                agg_ps = pagg.tile([P, d_agg], f32, tag="agg")
                # self-loop first: diag(dinv^2) @ local rows (no gather dep)
                dspi, dsc = divmod(diagcol[i], SE_CH)
                D_ap = spiece[dspi][:, dsc * P:(dsc + 1) * P]
                hloc = wp.tile([P, d_agg], bf16, tag="hloc")
                rows = P if i < NBLK - 1 else lastP
                if rows < P:
                    nc.vector.memset(hloc[:, :], 0.0)
                nc.sync.dma_start(out=hloc[:rows, :],
                                  in_=loc_ap[i * P:i * P + rows, :])
                nc.tensor.matmul(out=agg_ps[:], lhsT=D_ap, rhs=hloc[:],
                                 start=True, stop=False)
                pl = parts2[i]
                for j, (b, ch, col) in enumerate(pl):
                    spi, sc = divmod(col, SE_CH)
                    S_ap = spiece[spi][:, sc * P:(sc + 1) * P]
                    pi, c = divmod(ch, P_CH)
                    g = piece[(b, pi)]
                    nc.tensor.matmul(
                        out=agg_ps[:], lhsT=S_ap,
                        rhs=g[:, c * d_agg:(c + 1) * d_agg],
                        start=False, stop=(j == len(pl) - 1))
                conv_block(i, agg_ps, Wk, d_agg, d_out, csb, wp, ptr, pcv,
                           sum_ps, sq_ps, dscale=dvc[:, i:i + 1])
            nc.vector.tensor_copy(out=stat[li][0][:], in_=sum_ps[:])
            nc.vector.tensor_copy(out=stat[li][1][:], in_=sq_ps[:])

    # ---------------- layer 2
    csb2_pool = tc.alloc_tile_pool(name="csb2p", bufs=1)
    csb2 = csb2_pool.tile([P, NBLK * D_HID], bf16, name="csb2")
    gather_layer(ag1_out, ag1_in, D_HID, W2s, D_HID, csb2, 1)
    c2 = bn_cols(1, "g2", "beta2", D_HID)
    AGQ3 = 3
    qb3 = [round(q * NBLK / AGQ3) for q in range(AGQ3 + 1)]
    ag3_or = ag3_out[:, :].rearrange("(c r) d -> c (r d)", c=NCORES)
    with tc.tile_pool(name="p3b", bufs=3) as wp, \
         tc.tile_pool(name="p3bp", bufs=4, space="PSUM") as pp:
        for i in range(NBLK):
            p3 = pp.tile([P, D_OUT], f32, tag="p3")
            for k in range(D_HID // P):
                tp = pp.tile([P, P], bf16, tag="t1")
                nc.tensor.transpose(
                    out=tp[:],
                    in_=csb2[:, i * D_HID + k * P:i * D_HID + (k + 1) * P],
                    identity=eye[:])
                hT = wp.tile([P, P], bf16, tag="hT")
                nc.scalar.activation(out=hT[:], in_=tp[:], func=AF.Relu,
                                     scale=c2[k][:, 0:1], bias=c2[k][:, 1:2])
                nc.tensor.matmul(out=p3[:], lhsT=hT[:], rhs=W3s[k][:],
                                 start=(k == 0), stop=(k == D_HID // P - 1))
            c3sb = wp.tile([P, D_OUT], bf16, tag="c3")
            nc.vector.tensor_scalar(out=c3sb[:], in0=p3[:],
                                    scalar1=dvc[:, i:i + 1], scalar2=None,
                                    op0=OP.mult)
            rows = P if i < NBLK - 1 else lastP
            nc.sync.dma_start(out=ag3_in[i * P:i * P + rows, :],
                              in_=c3sb[:rows, :])
    csb2_pool.release()
    nc.gpsimd.collective_compute(
        "AllGather", mybir.AluOpType.bypass, RG,
        ins=[ag3_in[:, :]], outs=[ag3_out[:, :]])

    # ---------------- layer 3
    csb3_pool = tc.alloc_tile_pool(name="csb3p", bufs=1)
    csb3 = csb3_pool.tile([P, NBLK * D_OUT], bf16, name="csb3")
    gather_layer(ag3_out, ag3_in, D_OUT, None, D_OUT, csb3, 2)
    c3 = bn_cols(2, "g3", "beta3", D_OUT)
    with tc.tile_pool(name="p3c", bufs=3) as wp, \
         tc.tile_pool(name="p3cp", bufs=4, space="PSUM") as pp:
        for i in range(NBLK):
            tp = pp.tile([P, P], bf16, tag="t1")
            nc.tensor.transpose(out=tp[:], in_=csb3[:, i * P:(i + 1) * P],
                                identity=eye[:])
            xT = wp.tile([P, P], bf16, tag="xT")
            nc.scalar.activation(out=xT[:], in_=tp[:], func=AF.Relu,
                                 scale=c3[0][:, 0:1], bias=c3[0][:, 1:2])
            po = pp.tile([N_CLS, P], f32, tag="po")
            nc.tensor.matmul(out=po[:], lhsT=Wos[:], rhs=xT[:],
                             start=True, stop=True)
            osb = wp.tile([N_CLS, P], f32, tag="osb")
            nc.vector.tensor_scalar(out=osb[:], in0=po[:],
                                    scalar1=bocol[:, 0:1], scalar2=None,
                                    op0=OP.add)
            rows = P if i < NBLK - 1 else lastP
            nc.sync.dma_start(out=outT_d[:, i * P:i * P + rows],
                              in_=osb[:, :rows])
    csb3_pool.release()
    pers.release()
    dram.release()


# ------------------------------------------------------------------ kernel
def kernel(x, edge_index, W1, b1, g1, beta1, W2, b2, g2, beta2,
           W3, b3, g3, beta3, Wout, bout):
    import ml_dtypes
    import concourse.bass as bass
    import concourse.mybir as mybir
    from concourse.bacc import Bacc
    from concourse.tile import TileContext
    from concourse.bass_utils import run_bass_kernel_spmd

    x = np.asarray(x, dtype=np.float32)
    ei = np.asarray(edge_index, dtype=np.int64)
    N, D_IN = x.shape
    D_HID = np.asarray(W1).shape[1]
    D_OUT = np.asarray(W3).shape[1]
    N_CLS = np.asarray(Wout).shape[1]
    bf = ml_dtypes.bfloat16

    sched, dev = _host_prep(N, ei[0], ei[1], x, bf)

    nc = Bacc(num_devices=NCORES, num_swdge_queues=4)
    with TileContext(nc) as tc:
        _build(nc, tc, sched, N, D_IN, D_HID, D_OUT, N_CLS, bass, mybir)
    nc.compile()

    common = {
        "W1": np.asarray(W1, np.float32).astype(bf),
        "W2": np.asarray(W2, np.float32).astype(bf),
        "W3": np.asarray(W3, np.float32).astype(bf),
        "Wout": np.asarray(Wout, np.float32).astype(bf),
        "g1": np.asarray(g1, np.float32).reshape(1, -1),
        "beta1": np.asarray(beta1, np.float32).reshape(1, -1),
        "g2": np.asarray(g2, np.float32).reshape(1, -1),
        "beta2": np.asarray(beta2, np.float32).reshape(1, -1),
        "g3": np.asarray(g3, np.float32).reshape(1, -1),
        "beta3": np.asarray(beta3, np.float32).reshape(1, -1),
        "bout": np.asarray(bout, np.float32).reshape(-1, 1),
        "iota4": np.tile(np.arange(P, dtype=np.float32), (P, 4)).astype(bf),
        "ones": np.ones((P, 1), np.float32).astype(bf),
        "eye": np.eye(P, dtype=np.float32).astype(bf),
        "eyef": np.eye(2, dtype=np.float32),
    }
    in_maps = [{**common, **dev[c]} for c in range(NCORES)]
    _trace = bool(os.environ.get("GCN_TRACE"))
    res = run_bass_kernel_spmd(nc, in_maps, core_ids=list(range(NCORES)),
                               trace=_trace)
    out = np.concatenate([res.results[c]["outT"].T for c in range(NCORES)], axis=0)
    kernel._last_res = res
    return out


# revision 67
# speedup vs baseline: 1.0332x; 1.0079x over previous
"""GCN (3x GCNConv+BN+ReLU, linear head) on 8 TRN2 NeuronCores.

Strategy (graph/data parallel, dst-partitioned, bf16 data path):
  - Nodes row-sharded 8 ways; edges partitioned by dst core, grouped by
    dst block (128 nodes), per-block counts aligned across cores (~3% pad)
    so the SPMD program structure is identical on every core.
  - L1: NO device gathers. Host pre-builds an edge-major bf16 feature
    image (already in SBUF layout) streamed sequentially at full DMA
    bandwidth.
  - L2/L3: SWDGE dma_gather (4 queues, bf16 elems) of halo rows from the
    all-gathered activations; self-loops excluded from the gather stream
    and applied per block via a diagonal one-hot matmul reading the
    core-local rows with a sequential DMA.
  - Per 128-edge chunk: one-hot S (iota-compare, DVE, bf16) and a
    segment-sum matmul accumulating in PSUM per 128-node block. Chunks
    are densely packed; a chunk straddling two blocks is applied twice
    with disjoint norm masks.
  - BatchNorm: per-feature sums via ones-matmul + tiny AllReduce; conv
    outputs stay resident in SBUF; affine+ReLU applied in transposed
    layout as one ACT op per tile. Conv bias cancels in train-mode BN.
  - Activations all-gathered between layers in bf16.
"""
import os

import numpy as np

P = 128
NCORES = 8
BANKS = 4
P_CH = 8          # chunks per dma_gather piece
P_CH1 = 32        # chunks per L1 stream-load piece
SE_CH = 32        # S-tile cols per stream-load piece (L2/L3)
EPS = 1e-5


def _pack_idx(idx_flat):
    # [n] int16 -> [128, n//16] wrapped in 16 partitions, replicated x8
    return np.tile(idx_flat.reshape(-1, 16).T, (8, 1)).copy()


# ----------------------------------------------------------------- host prep
def _host_prep(N, src, dst, x32, bf):
    """Build per-core streams/tables + shared static schedule."""
    NPC = N // NCORES
    NBLK = (NPC + P - 1) // P
    BR = -(-N // BANKS)
    D_IN = x32.shape[1]

    loops = np.arange(N, dtype=np.int64)
    src_f = np.concatenate([src, loops])
    dst_f = np.concatenate([dst, loops])
    deg = np.bincount(dst_f, minlength=N).astype(np.float32)
    dinv = np.where(deg > 0, 1.0 / np.sqrt(deg), 0.0).astype(np.float32)
    w_f = (dinv[src_f] * dinv[dst_f]).astype(np.float32)

    # ---------- L1 stream: all edges incl self-loops, grouped by dst block
    core1 = dst_f // NPC
    per1 = []
    for c in range(NCORES):
        sel = np.flatnonzero(core1 == c)
        s, d, ww = src_f[sel], dst_f[sel] - c * NPC, w_f[sel]
        blk = d // P
        o = np.argsort(blk, kind="stable")
        per1.append((s[o], d[o], ww[o], blk[o]))
    cnt1 = np.zeros((NCORES, NBLK), np.int64)
    for c in range(NCORES):
        np.add.at(cnt1[c], per1[c][3], 1)
    m1 = cnt1.max(axis=0)                      # aligned per-block counts
    ofs1 = np.cumsum(m1) - m1
    L1 = int(m1.sum())
    nch1 = -(-L1 // P)
    L1p = nch1 * P

    cols1 = []                                  # (block, chunk)
    parts1 = [[] for _ in range(NBLK)]          # block -> [(chunk, col)]
    for i in range(NBLK):
        if m1[i] == 0:
            continue
        c0 = int(ofs1[i]) // P
        c1 = int(ofs1[i] + m1[i] - 1) // P
        for ch in range(c0, c1 + 1):
            parts1[i].append((ch, len(cols1)))
            cols1.append((i, ch))
    ncol1 = len(cols1)

    # ---------- L2/L3 streams: real edges only, by (dst block, src bank)
    core2 = dst // NPC
    w_r = w_f[: len(src)]
    per2 = []
    for c in range(NCORES):
        sel = np.flatnonzero(core2 == c)
        s, d, ww = src[sel], dst[sel] - c * NPC, w_r[sel]
        blk = d // P
        bank = s // BR
        o = np.argsort(blk * BANKS + bank, kind="stable")
        per2.append((s[o], d[o], ww[o], blk[o], bank[o]))
    cnt2 = np.zeros((NCORES, NBLK, BANKS), np.int64)
    for c in range(NCORES):
        np.add.at(cnt2[c], (per2[c][3], per2[c][4]), 1)
    m2 = cnt2.max(axis=0)                       # [NBLK, BANKS]
    ofs2 = np.cumsum(m2, axis=0) - m2           # stream offset per (blk,bank)
    L2 = m2.sum(axis=0)                         # [BANKS] stream lengths
    nch2 = [int(-(-max(int(L2[b]), 1) // P)) for b in range(BANKS)]

    cols2 = []                                  # (block, bank, chunk) | diag
    parts2 = [[] for _ in range(NBLK)]          # block -> [(bank, chunk, col)]
    diagcol = [0] * NBLK                        # block -> col of its diag tile
    for i in range(NBLK):
        diagcol[i] = len(cols2)
        cols2.append(("diag", i, -1))
        for b in range(BANKS):
            if m2[i, b] == 0:
                continue
            c0 = int(ofs2[i, b]) // P
            c1 = int(ofs2[i, b] + m2[i, b] - 1) // P
            for ch in range(c0, c1 + 1):
                parts2[i].append((b, ch, len(cols2)))
                cols2.append((i, b, ch))
    ncol2 = len(cols2)

    # piece emission schedule (stream pieces to load before block i)
    emit1 = [[] for _ in range(NBLK)]
    seen = set()
    for i in range(NBLK):
        for (ch, _) in parts1[i]:
            pi = ch // P_CH1
            if pi not in seen:
                seen.add(pi)
                emit1[i].append(pi)
    emitQ1 = [[] for _ in range(NBLK)]          # quad one-hot build schedule
    seen = set()
    for i in range(NBLK):
        for (_, col) in parts1[i]:
            qi = col // 4
            if qi not in seen:
                seen.add(qi)
                emitQ1[i].append(qi)
    emit2 = [[] for _ in range(NBLK)]
    seen = set()
    for i in range(NBLK):
        for (b, ch, _) in parts2[i]:
            pi = ch // P_CH
            if (b, pi) not in seen:
                seen.add((b, pi))
                emit2[i].append((b, pi))
    emitS2 = [[] for _ in range(NBLK)]
    seen = set()
    for i in range(NBLK):
        for col in [diagcol[i]] + [c for (_, _, c) in parts2[i]]:
            pi = col // SE_CH
            if pi not in seen:
                seen.add(pi)
                emitS2[i].append(pi)

    # ---------- per-core device arrays
    dev = []
    for c in range(NCORES):
        m = {}
        # L1: edge-major bf16 image [128, nch1*D_IN] + slot/norm cols
        s, d, ww, blk = per1[c]
        pos_in = np.zeros(L1p, np.int64)        # stream pos -> edge idx+1
        cur = 0
        for i in range(NBLK):
            n = int(cnt1[c][i])
            pos_in[int(ofs1[i]):int(ofs1[i]) + n] = np.arange(cur, cur + n) + 1
            cur += n
        srcs = np.zeros(L1p, np.int64)
        valid = pos_in > 0
        srcs[valid] = s[pos_in[valid] - 1]
        dloc = np.zeros(L1p, np.float32)
        wloc = np.zeros(L1p, np.float32)
        dloc[valid] = (d[pos_in[valid] - 1] % P).astype(np.float32)
        wloc[valid] = ww[pos_in[valid] - 1]
        # fold the edge norm into the streamed rows; S becomes a pure one-hot
        xe = (x32[srcs] * wloc[:, None]).astype(bf)   # [L1p, D_IN]
        m["xe"] = np.ascontiguousarray(
            xe.reshape(nch1, P, D_IN).transpose(1, 0, 2).reshape(P, nch1 * D_IN))
        blk_of = np.full(L1p, -1, np.int64)
        for i in range(NBLK):
            blk_of[int(ofs1[i]):int(ofs1[i] + m1[i])] = i
        # masked/pad entries get slot 255: matches no iota value
        slot1 = np.full((P, ncol1), 255.0, np.float32)
        for j, (i, ch) in enumerate(cols1):
            sl = slice(ch * P, (ch + 1) * P)
            mask = (blk_of[sl] == i) & valid[sl]
            slot1[:, j] = np.where(mask, dloc[sl], 255.0)
        m["slot1"] = slot1.astype(bf)

        # L2/L3: idx banks + slot/norm cols
        s, d, ww, blk, bank = per2[c]
        dl_all, w_all, blk_all = [], [], []
        for b in range(BANKS):
            Lb = nch2[b] * P
            idx = np.zeros(Lb, np.int16)
            dl = np.zeros(Lb, np.float32)
            wl = np.zeros(Lb, np.float32)
            bo = np.full(Lb, -1, np.int64)
            vv = np.zeros(Lb, bool)
            selb = bank == b
            sb, db, wb = s[selb], d[selb], ww[selb]
            cpos = 0
            for i in range(NBLK):
                n = int(cnt2[c][i, b])
                o0 = int(ofs2[i, b])
                idx[o0:o0 + n] = (sb[cpos:cpos + n] - b * BR).astype(np.int16)
                dl[o0:o0 + n] = (db[cpos:cpos + n] % P).astype(np.float32)
                wl[o0:o0 + n] = wb[cpos:cpos + n]
                vv[o0:o0 + n] = True
                bo[o0:o0 + int(m2[i, b])] = i
                cpos += n
            dl_all.append(dl)
            w_all.append(wl)
            blk_all.append((bo, vv))
            m[f"idx{b}"] = _pack_idx(idx)
        # per-node dinv columns [128, NBLK]: dinv[src] is folded into the
        # apply-phase row writes, dinv[dst] into the aggregate-copy scale,
        # so the S tiles are PURE one-hots (exactly representable in fp8).
        nod = c * NPC + np.arange(NBLK * P)
        dvl = np.zeros(NBLK * P, np.float32)
        ok = nod < (c + 1) * NPC
        dvl[ok] = dinv[nod[ok]]
        m["dvc"] = np.ascontiguousarray(dvl.reshape(NBLK, P).T)

        # masked/pad entries get slot 255: matches no iota value
        slot2 = np.full((P, max(ncol2, 1)), 255.0, np.float32)
        arp = np.arange(P, dtype=np.float32)
        for j, c3 in enumerate(cols2):
            if c3[0] == "diag":
                slot2[:, j] = arp
                continue
            i, b, ch = c3
            sl = slice(ch * P, (ch + 1) * P)
            bo, vv = blk_all[b]
            mask = (bo[sl] == i) & vv[sl]
            slot2[:, j] = np.where(mask, dl_all[b][sl], 255.0)
        # bake the one-hot S tiles [P, ncol2*P] fp8 (shared by L2 and L3;
        # diag cols come out as the identity via the same formula)
        import ml_dtypes as _mld
        S2 = (arp[None, None, :] == slot2[:, :, None])
        m["se2"] = np.ascontiguousarray(
            S2.reshape(P, -1).astype(_mld.float8_e4m3))
        dev.append(m)

    sched = dict(NPC=NPC, NBLK=NBLK, BR=BR, nch1=nch1, ncol1=ncol1,
                 parts1=parts1, emit1=emit1, emitQ1=emitQ1,
                 nch2=nch2, ncol2=ncol2, parts2=parts2, emit2=emit2,
                 emitS2=emitS2, diagcol=diagcol)
    return sched, dev


# ------------------------------------------------------------- device build
def _build(nc, tc, sched, N, D_IN, D_HID, D_OUT, N_CLS, bass, mybir):
    NPC, NBLK, BR = sched["NPC"], sched["NBLK"], sched["BR"]
    nch1, ncol1 = sched["nch1"], sched["ncol1"]
    parts1, emit1 = sched["parts1"], sched["emit1"]
    emitQ1 = sched["emitQ1"]
    nch2, ncol2 = sched["nch2"], sched["ncol2"]
    parts2, emit2 = sched["parts2"], sched["emit2"]
    emitS2, diagcol = sched["emitS2"], sched["diagcol"]
    f32 = mybir.dt.float32
    bf16 = mybir.dt.bfloat16
    AF = mybir.ActivationFunctionType
    OP = mybir.AluOpType
    RG = [list(range(NCORES))]
    lastP = NPC - (NBLK - 1) * P
    SP = not os.environ.get("GCN_NOSP")      # single_packet on gathers

    xe_d = nc.dram_tensor("xe", [P, nch1 * D_IN], bf16, kind="ExternalInput")
    W1_d = nc.dram_tensor("W1", [D_IN, D_HID], bf16, kind="ExternalInput")
    W2_d = nc.dram_tensor("W2", [D_HID, D_HID], bf16, kind="ExternalInput")
    W3_d = nc.dram_tensor("W3", [D_HID, D_OUT], bf16, kind="ExternalInput")
    Wo_d = nc.dram_tensor("Wout", [D_OUT, N_CLS], bf16, kind="ExternalInput")
    bn_d = {}
    for nm, dd in [("g1", D_HID), ("beta1", D_HID),
                   ("g2", D_HID), ("beta2", D_HID),
                   ("g3", D_OUT), ("beta3", D_OUT)]:
        bn_d[nm] = nc.dram_tensor(nm, [1, dd], f32, kind="ExternalInput")
    bo_d = nc.dram_tensor("bout", [N_CLS, 1], f32, kind="ExternalInput")
    iota4_d = nc.dram_tensor("iota4", [P, 4 * P], bf16, kind="ExternalInput")
    ones_d = nc.dram_tensor("ones", [P, 1], bf16, kind="ExternalInput")
    eye_d = nc.dram_tensor("eye", [P, P], bf16, kind="ExternalInput")
    eyef_d = nc.dram_tensor("eyef", [2, 2], f32, kind="ExternalInput")
    slot1_d = nc.dram_tensor("slot1", [P, ncol1], bf16, kind="ExternalInput")
    fp8 = mybir.dt.float8e4
    se2_d = nc.dram_tensor("se2", [P, max(ncol2, 1) * P], fp8,
                           kind="ExternalInput")
    dvc_d = nc.dram_tensor("dvc", [P, NBLK], f32, kind="ExternalInput")
    idx_d = [nc.dram_tensor(f"idx{b}", [P, max(nch2[b], 1) * 8], mybir.dt.int16,
                            kind="ExternalInput") for b in range(BANKS)]
    outT_d = nc.dram_tensor("outT", [N_CLS, NPC], f32, kind="ExternalOutput")

    pers = tc.alloc_tile_pool(name="pers", bufs=1)
    dram = tc.alloc_tile_pool(name="dram", bufs=1, space="DRAM")

    def ld(name, shape, dt_, src):
        t = pers.tile(shape, dt_, name=name)
        nc.sync.dma_start(out=t[:], in_=src)
        return t

    iota4 = ld("iota4", [P, 4 * P], bf16, iota4_d[:, :])
    dvc = ld("dvc", [P, NBLK], f32, dvc_d[:, :])
    ones = ld("ones", [P, 1], bf16, ones_d[:, :])
    eye = ld("eye", [P, P], bf16, eye_d[:, :])
    eyef = ld("eyef", [2, 2], f32, eyef_d[:, :])
    W1s = ld("W1s", [P, D_HID], bf16, W1_d[:, :])
    W2s = [ld(f"W2s{k}", [P, D_HID], bf16, W2_d[k * P:(k + 1) * P, :])
           for k in range(D_HID // P)]
    W3s = [ld(f"W3s{k}", [P, D_OUT], bf16, W3_d[k * P:(k + 1) * P, :])
           for k in range(D_HID // P)]
    Wos = ld("Wos", [P, N_CLS], bf16, Wo_d[:, :])
    bocol = ld("bocol", [N_CLS, 1], f32, bo_d[:, :])
    bnrow = {nm: ld(f"r_{nm}", [1, bn_d[nm].shape[1]], f32, bn_d[nm][:, :])
             for nm in bn_d}
    idx_s = [ld(f"idxs{b}", [P, max(nch2[b], 1) * 8], mybir.dt.int16,
                idx_d[b][:, :]) for b in range(BANKS)]

    ag1_in = dram.tile([NPC, D_HID], bf16, name="ag1_in")
    ag1_out = dram.tile([N, D_HID], bf16, name="ag1_out", addr_space="Shared")
    ag3_in = dram.tile([NPC, D_OUT], bf16, name="ag3_in")
    ag3_out = dram.tile([N, D_OUT], bf16, name="ag3_out", addr_space="Shared")

    stat = {}
    for li, dd in [(0, D_HID), (1, D_HID), (2, D_OUT)]:
        stat[li] = (pers.tile([1, dd], f32, name=f"ssum{li}"),
                    pers.tile([1, dd], f32, name=f"ssq{li}"))

    # --------------------------------------------------- shared inner pieces
    def conv_block(i, agg_ps, Wk, d_agg, d_out, csb, wp, ptr, pcv, sum_ps,
                   sq_ps, dscale=None):
        """agg psum -> (optional conv) -> csb[:, i*d_out:] + stats accum.
        dscale: per-partition dinv[dst] column folded out of the S tiles."""
        sc = dscale if dscale is not None else 1.0
        cslice = csb[:, i * d_out:(i + 1) * d_out]
        if Wk is None:
            nc.scalar.activation(out=cslice, in_=agg_ps[:], func=AF.Copy,
                                 scale=sc)
        else:
            agg_sb = wp.tile([P, d_agg], bf16, tag="aggsb")
            nc.scalar.activation(out=agg_sb[:], in_=agg_ps[:], func=AF.Copy,
                                 scale=sc)
            conv_ps = pcv.tile([P, d_out], f32, tag="conv")
            for k in range(d_agg // P):
                tp = ptr.tile([P, P], bf16, tag="tr")
                nc.tensor.transpose(out=tp[:], in_=agg_sb[:, k * P:(k + 1) * P],
                                    identity=eye[:])
                tsb = wp.tile([P, P], bf16, tag="aggT")
                nc.vector.tensor_copy(out=tsb[:], in_=tp[:])
                nc.tensor.matmul(out=conv_ps[:], lhsT=tsb[:], rhs=Wk[k][:],
                                 start=(k == 0), stop=(k == d_agg // P - 1))
            nc.scalar.activation(out=cslice, in_=conv_ps[:], func=AF.Copy)
        sq = wp.tile([P, d_out], bf16, tag="sq")
        nc.scalar.activation(out=sq[:], in_=cslice, func=AF.Square)
        nc.tensor.matmul(out=sum_ps[:], lhsT=ones[:], rhs=cslice,
                         start=(i == 0), stop=(i == NBLK - 1))
        nc.tensor.matmul(out=sq_ps[:], lhsT=ones[:], rhs=sq[:],
                         start=(i == 0), stop=(i == NBLK - 1))

    def bn_cols(li, gname, bename, d_out):
        stat_sum, stat_sq = stat[li]
        ar_in = dram.tile([2, d_out], f32, name=f"arin{li}")
        ar_out = dram.tile([2, d_out], f32, name=f"arout{li}", addr_space="Shared")
        nc.sync.dma_start(out=ar_in[0:1, :], in_=stat_sum[:])
        nc.sync.dma_start(out=ar_in[1:2, :], in_=stat_sq[:])
        nc.gpsimd.collective_compute(
            "AllReduce", OP.add, RG, ins=[ar_in[:, :]], outs=[ar_out[:, :]])
        with tc.tile_pool(name=f"bn{li}", bufs=1) as bp, \
             tc.tile_pool(name=f"bnp{li}", bufs=1, space="PSUM") as bpp:
            st0 = bp.tile([1, d_out], f32, name=f"st0{li}")
            nc.sync.dma_start(out=st0[:], in_=ar_out[0:1, :])
            st1 = bp.tile([1, d_out], f32, name=f"st1{li}")
            nc.sync.dma_start(out=st1[:], in_=ar_out[1:2, :])
            mean = bp.tile([1, d_out], f32, name=f"mean{li}")
            nc.vector.tensor_scalar(out=mean[:], in0=st0[:], scalar1=1.0 / N,
                                    scalar2=None, op0=OP.mult)
            ex2 = bp.tile([1, d_out], f32, name=f"ex2{li}")
            nc.vector.tensor_scalar(out=ex2[:], in0=st1[:], scalar1=1.0 / N,
                                    scalar2=None, op0=OP.mult)
            m2t = bp.tile([1, d_out], f32, name=f"m2{li}")
            nc.vector.tensor_tensor(out=m2t[:], in0=mean[:], in1=mean[:],
                                    op=OP.mult)
            var = bp.tile([1, d_out], f32, name=f"var{li}")
            nc.vector.tensor_tensor(out=var[:], in0=ex2[:], in1=m2t[:],
                                    op=OP.subtract)
            nc.vector.tensor_scalar(out=var[:], in0=var[:], scalar1=EPS,
                                    scalar2=None, op0=OP.add)
            sd = bp.tile([1, d_out], f32, name=f"sd{li}")
            nc.scalar.activation(out=sd[:], in_=var[:], func=AF.Sqrt)
            rinv = bp.tile([1, d_out], f32, name=f"rinv{li}")
            nc.vector.reciprocal(out=rinv[:], in_=sd[:])
            scl = bp.tile([1, d_out], f32, name=f"scl{li}")
            nc.vector.tensor_tensor(out=scl[:], in0=rinv[:],
                                    in1=bnrow[gname][:], op=OP.mult)
            tmp = bp.tile([1, d_out], f32, name=f"tmp{li}")
            nc.vector.tensor_tensor(out=tmp[:], in0=mean[:], in1=scl[:],
                                    op=OP.mult)
            sht = bp.tile([1, d_out], f32, name=f"sht{li}")
            nc.vector.tensor_tensor(out=sht[:], in0=bnrow[bename][:],
                                    in1=tmp[:], op=OP.subtract)
            pk = bp.tile([2, d_out], f32, name=f"pk{li}")
            nc.sync.dma_start(out=pk[0:1, :], in_=scl[:])
            nc.sync.dma_start(out=pk[1:2, :], in_=sht[:])
            cols = []
            for k in range(d_out // P):
                tp = bpp.tile([P, 2], f32, name=f"bnt{li}_{k}")
                nc.tensor.transpose(out=tp[:], in_=pk[:, k * P:(k + 1) * P],
                                    identity=eyef[:])
                cc = pers.tile([P, 2], f32, name=f"bncol{li}_{k}")
                nc.vector.tensor_copy(out=cc[:], in_=tp[:])
                cols.append(cc)
        return cols

    # ------------------------------------------------------------ L1 phase
    csb1_pool = tc.alloc_tile_pool(name="csb1p", bufs=1)
    csb1 = csb1_pool.tile([P, NBLK * D_HID], bf16, name="csb1")
    slot1_s = None
    with tc.tile_pool(name="l1t", bufs=1) as l1t, \
         tc.tile_pool(name="g1", bufs=3) as gp, \
         tc.tile_pool(name="w1p", bufs=3) as wp, \
         tc.tile_pool(name="s1p", bufs=4) as spool, \
         tc.tile_pool(name="ps1", bufs=2, space="PSUM") as pagg, \
         tc.tile_pool(name="pt1", bufs=2, space="PSUM") as ptr, \
         tc.tile_pool(name="pc1", bufs=2, space="PSUM") as pcv, \
         tc.tile_pool(name="pst1", bufs=1, space="PSUM") as pst:
        slot1_s = l1t.tile([P, ncol1], bf16, name="slot1s")
        nc.sync.dma_start(out=slot1_s[:], in_=slot1_d[:, :])
        sum_ps = pst.tile([1, D_HID], f32, name="sum1")
        sq_ps = pst.tile([1, D_HID], f32, name="sq1")
        piece = {}
        squad = {}
        for i in range(NBLK):
            for pi in emit1[i]:
                nchp = min(P_CH1, nch1 - pi * P_CH1)
                g = gp.tile([P, P_CH1 * D_IN], bf16, tag="gx", bufs=3,
                            name=f"gx{pi}")
                nc.sync.dma_start(
                    out=g[:, :nchp * D_IN],
                    in_=xe_d[:, pi * P_CH1 * D_IN:(pi * P_CH1 + nchp) * D_IN])
                piece[pi] = g
            for qi in emitQ1[i]:
                nq = min(4, ncol1 - qi * 4)
                S4 = spool.tile([P, 4 * P], bf16, tag="S4", bufs=6,
                                name=f"S4_{qi}")
                nc.vector.tensor_tensor(
                    out=S4[:, :nq * P].rearrange("p (c s) -> p c s", s=P),
                    in0=iota4[:, :nq * P].rearrange("p (c s) -> p c s", s=P),
                    in1=slot1_s[:, qi * 4:qi * 4 + nq].to_broadcast([P, nq, P]),
                    op=OP.is_equal)
                squad[qi] = S4
            agg_ps = pagg.tile([P, D_IN], f32, tag="agg")
            pl = parts1[i]
            for j, (ch, col) in enumerate(pl):
                qi, qc = divmod(col, 4)
                S_ap = squad[qi][:, qc * P:(qc + 1) * P]
                pi, c = divmod(ch, P_CH1)
                g = piece[pi]
                nc.tensor.matmul(
                    out=agg_ps[:], lhsT=S_ap,
                    rhs=g[:, c * D_IN:(c + 1) * D_IN],
                    start=(j == 0), stop=(j == len(pl) - 1))
            conv_block(i, agg_ps, [W1s], D_IN, D_HID, csb1, wp, ptr, pcv,
                       sum_ps, sq_ps)
        nc.vector.tensor_copy(out=stat[0][0][:], in_=sum_ps[:])
        nc.vector.tensor_copy(out=stat[0][1][:], in_=sq_ps[:])

    # ---------------- L1 apply -> ag1 (partial AllGathers overlap apply)
    c1 = bn_cols(0, "g1", "beta1", D_HID)
    AGQ1 = 4
    qb1 = [round(q * NBLK / AGQ1) for q in range(AGQ1 + 1)]
    ag1_or = ag1_out[:, :].rearrange("(c r) d -> c (r d)", c=NCORES)
    with tc.tile_pool(name="p3a", bufs=3) as wp, \
         tc.tile_pool(name="p3ap", bufs=4, space="PSUM") as pp:
        for i in range(NBLK):
            hrow = wp.tile([P, D_HID], bf16, tag="hrow")
            for k in range(D_HID // P):
                tp = pp.tile([P, P], bf16, tag="t1")
                nc.tensor.transpose(
                    out=tp[:],
                    in_=csb1[:, i * D_HID + k * P:i * D_HID + (k + 1) * P],
                    identity=eye[:])
                hT = wp.tile([P, P], bf16, tag="hT")
                nc.scalar.activation(out=hT[:], in_=tp[:], func=AF.Relu,
                                     scale=c1[k][:, 0:1], bias=c1[k][:, 1:2])
                tp2 = pp.tile([P, P], bf16, tag="t2")
                nc.tensor.transpose(out=tp2[:], in_=hT[:], identity=eye[:])
                nc.vector.tensor_scalar(
                    out=hrow[:, k * P:(k + 1) * P], in0=tp2[:],
                    scalar1=dvc[:, i:i + 1], scalar2=None, op0=OP.mult)
            rows = P if i < NBLK - 1 else lastP
            nc.sync.dma_start(out=ag1_in[i * P:i * P + rows, :],
                              in_=hrow[:rows, :])
    csb1_pool.release()
    nc.gpsimd.collective_compute(
        "AllGather", mybir.AluOpType.bypass, RG,
        ins=[ag1_in[:, :]], outs=[ag1_out[:, :]])

    # ------------------------------------------------- gather-based layer
    def gather_layer(table_ap, loc_ap, d_agg, Wk, d_out, csb, li):
        with tc.tile_pool(name=f"g{li}", bufs=3) as gp, \
             tc.tile_pool(name=f"w{li}", bufs=3) as wp, \
             tc.tile_pool(name=f"s{li}", bufs=4) as spool, \
             tc.tile_pool(name=f"ps{li}", bufs=2, space="PSUM") as pagg, \
             tc.tile_pool(name=f"pt{li}", bufs=2, space="PSUM") as ptr, \
             tc.tile_pool(name=f"pc{li}", bufs=2, space="PSUM") as pcv, \
             tc.tile_pool(name=f"pst{li}", bufs=1, space="PSUM") as pst:
            sum_ps = pst.tile([1, d_out], f32, name=f"sum{li}")
            sq_ps = pst.tile([1, d_out], f32, name=f"sq{li}")
            piece = {}
            spiece = {}
            for i in range(NBLK):
                for pi in emitS2[i]:
                    ncolp = min(SE_CH, ncol2 - pi * SE_CH)
                    se = spool.tile([P, SE_CH * P], fp8, tag="se", bufs=3,
                                    name=f"se{li}_{pi}")
                    nc.sync.dma_start(
                        out=se[:, :ncolp * P],
                        in_=se2_d[:, pi * SE_CH * P:(pi * SE_CH + ncolp) * P])
                    spiece[pi] = se
                for (b, pi) in emit2[i]:
                    cb0 = pi * P_CH
                    nchp = min(P_CH, nch2[b] - cb0)
                    if nchp <= 0:
                        continue
                    g = gp.tile([P, P_CH * d_agg], bf16, tag=f"gb{b}", bufs=3,
                                name=f"g{li}_{b}_{pi}")
                    nrows = min(BR, N - b * BR)
                    nc.gpsimd.dma_gather(
                        out_ap=g[:, :nchp * d_agg].rearrange(
                            "p (c d) -> p c d", d=d_agg),
                        in_ap=table_ap[b * BR:b * BR + nrows, :],
                        idxs_ap=idx_s[b][:, cb0 * 8:(cb0 + nchp) * 8],
                        num_idxs=nchp * P,
                        num_idxs_reg=nchp * P,
                        elem_size=d_agg,
                        single_packet=SP,
                        queue_num=b,
                    )
                    piece[(b, pi)] = g
                agg_ps = pagg.tile([P, d_agg], f32, tag="agg")
                # self-loop first: diag(dinv^2) @ local rows (no gather dep)
                dspi, dsc = divmod(diagcol[i], SE_CH)
                D_ap = spiece[dspi][:, dsc * P:(dsc + 1) * P]
                hloc = wp.tile([P, d_agg], bf16, tag="hloc")
                rows = P if i < NBLK - 1 else lastP
                if rows < P:
                    nc.vector.memset(hloc[:, :], 0.0)
                nc.sync.dma_start(out=hloc[:rows, :],
                                  in_=loc_ap[i * P:i * P + rows, :])
                nc.tensor.matmul(out=agg_ps[:], lhsT=D_ap, rhs=hloc[:],
                                 start=True, stop=False)
                pl = parts2[i]
                for j, (b, ch, col) in enumerate(pl):
                    spi, sc = divmod(col, SE_CH)
                    S_ap = spiece[spi][:, sc * P:(sc + 1) * P]
                    pi, c = divmod(ch, P_CH)
                    g = piece[(b, pi)]
                    nc.tensor.matmul(
                        out=agg_ps[:], lhsT=S_ap,
                        rhs=g[:, c * d_agg:(c + 1) * d_agg],
                        start=False, stop=(j == len(pl) - 1))
                conv_block(i, agg_ps, Wk, d_agg, d_out, csb, wp, ptr, pcv,
                           sum_ps, sq_ps, dscale=dvc[:, i:i + 1])
            nc.vector.tensor_copy(out=stat[li][0][:], in_=sum_ps[:])
            nc.vector.tensor_copy(out=stat[li][1][:], in_=sq_ps[:])

    # ---------------- layer 2
    csb2_pool = tc.alloc_tile_pool(name="csb2p", bufs=1)
    csb2 = csb2_pool.tile([P, NBLK * D_HID], bf16, name="csb2")
    gather_layer(ag1_out, ag1_in, D_HID, W2s, D_HID, csb2, 1)
    c2 = bn_cols(1, "g2", "beta2", D_HID)
    AGQ3 = 3
    qb3 = [round(q * NBLK / AGQ3) for q in range(AGQ3 + 1)]
    ag3_or = ag3_out[:, :].rearrange("(c r) d -> c (r d)", c=NCORES)
    with tc.tile_pool(name="p3b", bufs=3) as wp, \
         tc.tile_pool(name="p3bp", bufs=4, space="PSUM") as pp:
        for i in range(NBLK):
            p3 = pp.tile([P, D_OUT], f32, tag="p3")
            for k in range(D_HID // P):
                tp = pp.tile([P, P], bf16, tag="t1")
                nc.tensor.transpose(
                    out=tp[:],
                    in_=csb2[:, i * D_HID + k * P:i * D_HID + (k + 1) * P],
                    identity=eye[:])
                hT = wp.tile([P, P], bf16, tag="hT")
                nc.scalar.activation(out=hT[:], in_=tp[:], func=AF.Relu,
                                     scale=c2[k][:, 0:1], bias=c2[k][:, 1:2])
                nc.tensor.matmul(out=p3[:], lhsT=hT[:], rhs=W3s[k][:],
                                 start=(k == 0), stop=(k == D_HID // P - 1))
            c3sb = wp.tile([P, D_OUT], bf16, tag="c3")
            nc.vector.tensor_scalar(out=c3sb[:], in0=p3[:],
                                    scalar1=dvc[:, i:i + 1], scalar2=None,
                                    op0=OP.mult)
            rows = P if i < NBLK - 1 else lastP
            nc.sync.dma_start(out=ag3_in[i * P:i * P + rows, :],
                              in_=c3sb[:rows, :])
    csb2_pool.release()
    nc.gpsimd.collective_compute(
        "AllGather", mybir.AluOpType.bypass, RG,
        ins=[ag3_in[:, :]], outs=[ag3_out[:, :]])

    # ---------------- layer 3
    csb3_pool = tc.alloc_tile_pool(name="csb3p", bufs=1)
    csb3 = csb3_pool.tile([P, NBLK * D_OUT], bf16, name="csb3")
    gather_layer(ag3_out, ag3_in, D_OUT, None, D_OUT, csb3, 2)
    c3 = bn_cols(2, "g3", "beta3", D_OUT)
    with tc.tile_pool(name="p3c", bufs=3) as wp, \
         tc.tile_pool(name="p3cp", bufs=4, space="PSUM") as pp:
        for i in range(NBLK):
            tp = pp.tile([P, P], bf16, tag="t1")
            nc.tensor.transpose(out=tp[:], in_=csb3[:, i * P:(i + 1) * P],
                                identity=eye[:])
            xT = wp.tile([P, P], bf16, tag="xT")
            nc.scalar.activation(out=xT[:], in_=tp[:], func=AF.Relu,
                                 scale=c3[0][:, 0:1], bias=c3[0][:, 1:2])
            po = pp.tile([N_CLS, P], f32, tag="po")
            nc.tensor.matmul(out=po[:], lhsT=Wos[:], rhs=xT[:],
                             start=True, stop=True)
            osb = wp.tile([N_CLS, P], f32, tag="osb")
            nc.vector.tensor_scalar(out=osb[:], in0=po[:],
                                    scalar1=bocol[:, 0:1], scalar2=None,
                                    op0=OP.add)
            rows = P if i < NBLK - 1 else lastP
            nc.sync.dma_start(out=outT_d[:, i * P:i * P + rows],
                              in_=osb[:, :rows])
    csb3_pool.release()
    pers.release()
    dram.release()


# ------------------------------------------------------------------ kernel
def kernel(x, edge_index, W1, b1, g1, beta1, W2, b2, g2, beta2,
           W3, b3, g3, beta3, Wout, bout):
    import ml_dtypes
    import concourse.bass as bass
    import concourse.mybir as mybir
    from concourse.bacc import Bacc
    from concourse.tile import TileContext
    from concourse.bass_utils import run_bass_kernel_spmd

    x = np.asarray(x, dtype=np.float32)
    ei = np.asarray(edge_index, dtype=np.int64)
    N, D_IN = x.shape
    D_HID = np.asarray(W1).shape[1]
    D_OUT = np.asarray(W3).shape[1]
    N_CLS = np.asarray(Wout).shape[1]
    bf = ml_dtypes.bfloat16

    sched, dev = _host_prep(N, ei[0], ei[1], x, bf)

    nc = Bacc(num_devices=NCORES, num_swdge_queues=4)
    with TileContext(nc) as tc:
        _build(nc, tc, sched, N, D_IN, D_HID, D_OUT, N_CLS, bass, mybir)
    nc.compile()

    common = {
        "W1": np.asarray(W1, np.float32).astype(bf),
        "W2": np.asarray(W2, np.float32).astype(bf),
        "W3": np.asarray(W3, np.float32).astype(bf),
        "Wout": np.asarray(Wout, np.float32).astype(bf),
        "g1": np.asarray(g1, np.float32).reshape(1, -1),
        "beta1": np.asarray(beta1, np.float32).reshape(1, -1),
        "g2": np.asarray(g2, np.float32).reshape(1, -1),
        "beta2": np.asarray(beta2, np.float32).reshape(1, -1),
        "g3": np.asarray(g3, np.float32).reshape(1, -1),
        "beta3": np.asarray(beta3, np.float32).reshape(1, -1),
        "bout": np.asarray(bout, np.float32).reshape(-1, 1),
        "iota4": np.tile(np.arange(P, dtype=np.float32), (P, 4)).astype(bf),
        "ones": np.ones((P, 1), np.float32).astype(bf),
        "eye": np.eye(P, dtype=np.float32).astype(bf),
        "eyef": np.eye(2, dtype=np.float32),
    }
    in_maps = [{**common, **dev[c]} for c in range(NCORES)]
    _trace = bool(os.environ.get("GCN_TRACE"))
    res = run_bass_kernel_spmd(nc, in_maps, core_ids=list(range(NCORES)),
                               trace=_trace)
    out = np.concatenate([res.results[c]["outT"].T for c in range(NCORES)], axis=0)
    kernel._last_res = res
    return out
